# revision 7
# baseline (speedup 1.0000x reference)
"""Trainium2 Bass kernel for nn_Diff_prop_18425409699925 (GNN message passing).

Math (per batch element b, with x = local_feat[b] reshaped to [n=1024, c=256]):
  xn   = x / ||x||_row
  A    = (xn * diag(W_adj)) @ xn^T                (symmetric; einsum uses only
                                                   the diagonal of W_adj)
  G    = exp(5*A) with diagonal zeroed            (the reference's row-max
                                                   shift cancels exactly in the
                                                   row-normalized mean)
  M    = (G @ x) / rowsum(G)
  diff = (x - M) @ W_aff^T + b_aff
  y    = LeakyReLU(LayerNorm(diff) * gamma + beta, 0.01)

Sharding: data-parallel over batch B=8 -> one batch element per NeuronCore,
weights replicated, no collectives. G (symmetric) is used directly as the
lhsT of the G @ x matmul, avoiding a [1024,1024] transpose.

All matmuls and big elementwise ops run in bf16 (operands; PSUM accumulation
is fp32).  Every ScalarE activation in the kernel (Exp, Ln, Square, Copy,
Prelu) lives in the single `natural_log_exp_and_others` table set, so the
~2.7us ACT table load is paid exactly once; sqrt/rsqrt/reciprocal are
computed as exp(-k*ln(s)).

The row-sum of diff (needed for the LayerNorm mean) is obtained for free by
appending the column-sums of W as an extra matmul column.  The sign of W is
flipped host-side so the on-device D' = M - x (natural STT operand order)
yields diff = (x - M) @ W^T.

global_feat and pos are unused by the reference; accepted and ignored.
"""

import os
import sys

import numpy as np

for _p in ("/opt/trn_rl_repo",):
    if os.path.isdir(_p) and _p not in sys.path:
        sys.path.insert(0, _p)

import ml_dtypes
import concourse.bacc as bacc
import concourse.bass as bass
import concourse.tile as tile
from concourse import mybir
from concourse.bass_utils import run_bass_kernel_spmd

B, T, NN, C = 8, 16, 64, 256
N = T * NN            # 1024 nodes per batch element
P = 128               # partitions
NT = N // P           # 8 n-tiles
CT = C // P           # 2 c-tiles
CX = C + 1            # D@W output incl. the rowsum column
F32 = mybir.dt.float32
BF16 = mybir.dt.bfloat16
TS = bass.ts
BF = ml_dtypes.bfloat16

LN_EPS = 1e-5
LEAK = 0.01
DIAG_NEG = -200.0     # added to diagonal of A pre-exp -> exp underflows to 0


def _steered_act_tables(orig_fn):
    """Steer the ACT table-set chooser: Exp and Ln both live in
    `natural_log_exp_and_others`, but the greedy chooser maps each function
    to the FIRST set containing it (natural_log for Ln, exp_and_others for
    Exp), thrashing a ~2.7us table load on every Ln<->Exp alternation.
    Hiding Exp/Ln from every other set makes the chooser settle on the one
    set that contains all activations this kernel uses -> exactly one load.
    Set indices (and hence the emitted act_func_set_id) are unchanged."""
    def wrapped(arch):
        t = orig_fn(arch)
        AF = mybir.ActivationFunctionType
        if "natural_log_exp_and_others" in t:
            for nm in t:
                if nm != "natural_log_exp_and_others":
                    t[nm] = t[nm] - {AF.Exp, AF.Ln}
        return t
    return wrapped


def _build_program(diag_one, ln_trivial):
    nc = bacc.Bacc("TRN2", target_bir_lowering=False, debug=False)
    _orig_gat = bacc.get_activation_tables
    bacc.get_activation_tables = _steered_act_tables(_orig_gat)
    try:
        return _build_program_inner(nc, diag_one, ln_trivial)
    finally:
        bacc.get_activation_tables = _orig_gat


def _build_program_inner(nc, diag_one, ln_trivial):

    xb_d = nc.declare_dram_parameter("xb", [N, C], BF16, isOutput=False)
    wt_d = nc.declare_dram_parameter("wt", [C, CX], BF16, isOutput=False)
    identb_d = nc.declare_dram_parameter("identb", [P, P], BF16, isOutput=False)
    wdiag_d = nc.declare_dram_parameter("wdiag", [P, CT], F32, isOutput=False)
    rows_d = nc.declare_dram_parameter("rows", [1, 2 * C], F32, isOutput=False)
    bvec_d = nc.declare_dram_parameter("bvec", [1, CX], BF16, isOutput=False)
    y_d = nc.declare_dram_parameter("y", [N, C], F32, isOutput=True)

    with tile.TileContext(nc) as tc:
        _emit(nc, tc, xb_d, wt_d, identb_d, wdiag_d, rows_d, bvec_d, y_d,
              diag_one, ln_trivial)
    nc.finalize()
    return nc


def _emit(nc, tc, xb_d, wt_d, identb_d, wdiag_d, rows_d, bvec_d, y_d,
          diag_one, ln_trivial):
    from contextlib import ExitStack

    mult = mybir.AluOpType.mult
    add = mybir.AluOpType.add
    subtract = mybir.AluOpType.subtract
    bypass = mybir.AluOpType.bypass
    amax = mybir.AluOpType.max
    AF = mybir.ActivationFunctionType

    W0 = int(os.environ.get("KERNEL_WARMUP", "48"))
    WA = int(os.environ.get("KERNEL_WARMUP_A", "2"))
    use_prelu = bool(int(os.environ.get("KERNEL_PRELU", "1")))

    v = nc.vector
    s = nc.scalar
    te = nc.tensor
    sy = nc.sync

    with ExitStack() as ctx:
        sb = ctx.enter_context(tc.tile_pool(name="sb", bufs=1))
        scr = ctx.enter_context(tc.tile_pool(name="scr", bufs=3))
        ps_a = ctx.enter_context(tc.tile_pool(name="ps_a", bufs=2, space="PSUM"))
        ps_c = ctx.enter_context(tc.tile_pool(name="ps_c", bufs=3, space="PSUM"))
        ps_t = ctx.enter_context(tc.tile_pool(name="ps_t", bufs=1, space="PSUM"))

        # ---------------- persistent SBUF tiles ----------------
        Xb = sb.tile([P, NT, C], BF16, tag="Xb", name="Xb")
        xnT = sb.tile([P, CT, N], BF16, tag="xnT", name="xnT")
        if diag_one:
            xnTs = xnT
        else:
            xnTs = sb.tile([P, CT, N], BF16, tag="xnTs", name="xnTs")
        G = [sb.tile([P, N], BF16, tag=f"G{i}", name=f"G{i}") for i in range(NT)]
        DTt = sb.tile([P, CT, N], BF16, tag="DTt", name="DTt")
        Y = sb.tile([P, NT, C], F32, tag="Y", name="Y")
        WT = sb.tile([P, CT, CX], BF16, tag="WT", name="WT")
        identb = sb.tile([P, P], BF16, tag="identb", name="identb")
        negeye = sb.tile([P, P], F32, tag="negeye", name="negeye")
        warm_src = sb.tile([P, C], BF16, tag="warm_src", name="warm_src")
        eps = sb.tile([P, 1], F32, tag="eps", name="eps")

        # batched per-row stats, one column per n-tile
        def stat(nm):
            return sb.tile([P, NT], F32, tag=nm, name=nm)
        SS, LNS, RNO = stat("SS"), stat("LNS"), stat("RNO")
        SP, LSP, NRS = stat("SP"), stat("LSP"), stat("NRS")
        SQ, MU, MUSQ = stat("SQ"), stat("MU"), stat("MUSQ")
        VAR, LV, RSTD, NB = stat("VAR"), stat("LV"), stat("RSTD"), stat("NB")

        if not diag_one:
            wdiag = sb.tile([P, CT], F32, tag="wdiag", name="wdiag")
        if not ln_trivial:
            g_bc = sb.tile([P, C], F32, tag="g_bc", name="g_bc")
            be_bc = sb.tile([P, C], F32, tag="be_bc", name="be_bc")
            rows = sb.tile([1, 2 * C], F32, tag="rows", name="rows")
            bvec = sb.tile([1, CX], BF16, tag="bvec", name="bvec")
            ones1f = sb.tile([1, P], F32, tag="ones1f", name="ones1f")
            ones1b = sb.tile([1, P], BF16, tag="ones1b", name="ones1b")

        # ---------------- loads (sync/HWDGE queues) ----------------
        sy.dma_start(identb[:], identb_d[:])
        wt_g = wt_d[:].rearrange("(k p) x -> p k x", p=P)
        sy.dma_start(WT[:], wt_g)
        if not diag_one:
            sy.dma_start(wdiag[:], wdiag_d[:])
        if not ln_trivial:
            sy.dma_start(rows[:], rows_d[:])
            sy.dma_start(bvec[:], bvec_d[:])
        xg = xb_d[:].rearrange("(i p) c -> p i c", p=P)
        for i in range(NT):
            sy.dma_start(Xb[:, i, :], xg[:, i, :])

        v.memset(eps[:], LN_EPS)
        v.memset(warm_src[:], 0.0)
        v.tensor_scalar_mul(negeye[:], identb[:], DIAG_NEG)

        if not ln_trivial:
            v.memset(ones1f[:], 1.0)
            v.memset(ones1b[:], 1.0)
            pg = ps_a.tile([P, N], F32, tag="pa", name="pg")
            nc.tensor.matmul(pg[:, 0:2 * C], ones1f[:], rows[:],
                             start=True, stop=True)
            v.tensor_copy(g_bc[:], pg[:, 0:C])
            v.tensor_copy(be_bc[:], pg[:, C:2 * C])

        # PE warm-up: keep the HAM clock gate open during the input-DMA
        # ramp so phase-B matmuls run at 2.4 GHz from the start.
        pw = None
        if W0 or WA:
            pw = ps_a.tile([P, N], F32, tag="pa", name="pw")
            for _ in range(W0):
                te.matmul(pw[:, 0:C], warm_src[:, 0:P], warm_src[:],
                          start=True, stop=True)

        # ---------------- phase A: row-normalize, build xn^T ----------------
        for i in range(NT):
            sqs = scr.tile([P, C], BF16, tag="sqs", name="sqs")
            v.scalar_tensor_tensor(
                out=sqs[:], in0=Xb[:, i, :], scalar=1.0, in1=Xb[:, i, :],
                op0=bypass, op1=mult, accum_out=SS[:, i:i + 1])
            s.activation(LNS[:, i:i + 1], SS[:, i:i + 1], AF.Ln)
            s.activation(RNO[:, i:i + 1], LNS[:, i:i + 1], AF.Exp, scale=-0.5)
            xn = scr.tile([P, C], BF16, tag="xn", name="xn")
            v.tensor_scalar_mul(xn[:], Xb[:, i, :], RNO[:, i:i + 1])
            pt = ps_t.tile([P, CT, P], BF16, tag="pt", name="pt")
            for k in range(CT):
                te.transpose(pt[:, k, :], xn[:, TS(k, P)], identb[:])
            if pw is not None:
                for _ in range(WA):
                    te.matmul(pw[:, 0:C], warm_src[:, 0:P], warm_src[:],
                              start=True, stop=True)
            dst = xnT[:, :, TS(i, P)]
            if i % 2 == 0:
                s.activation(dst, pt[:], AF.Copy)
            else:
                v.tensor_copy(dst, pt[:])
            if not diag_one:
                for k in range(CT):
                    v.tensor_scalar_mul(
                        xnTs[:, k, TS(i, P)], pt[:, k, :], wdiag[:, k:k + 1])

        # ---------------- phase B: A = xnTs^T @ xnT, G = exp(5A) ----------------
        for i in range(NT):
            pa = ps_a.tile([P, N], F32, tag="pa", name=f"pa{i}")
            for j in range(2):
                for k in range(CT):
                    te.matmul(
                        pa[:, TS(j, 512)],
                        xnTs[:, k, TS(i, P)],
                        xnT[:, k, TS(j, 512)],
                        start=(k == 0), stop=(k == CT - 1))
            v.tensor_add(pa[:, TS(i, P)], pa[:, TS(i, P)], negeye[:])
            s.activation(G[i][:], pa[:], AF.Exp, scale=5.0,
                         accum_out=SP[:, i:i + 1])
        s.activation(LSP[:], SP[:], AF.Ln)
        s.activation(NRS[:], LSP[:], AF.Exp, scale=-1.0)

        # ---------------- phase C: M, diff = (x-M)W^T, LN, LeakyReLU ----------------
        y_g = y_d[:].rearrange("(i p) c -> p i c", p=P)
        for i in range(NT):
            py = ps_c.tile([P, CX], F32, tag="pc", name=f"py{i}")
            for k in range(NT):
                te.matmul(py[:, 0:C], G[k][:, TS(i, P)], Xb[:, k, :],
                          start=(k == 0), stop=(k == NT - 1))
            # D' = M - x  (sign of W is pre-flipped so diff comes out right)
            Db = scr.tile([P, C], BF16, tag="db", name="db")
            v.scalar_tensor_tensor(
                out=Db[:], in0=py[:, 0:C], scalar=NRS[:, i:i + 1],
                in1=Xb[:, i, :], op0=mult, op1=subtract)
            ptd = ps_t.tile([P, CT, P], BF16, tag="pt", name="ptd")
            for k in range(CT):
                te.transpose(ptd[:, k, :], Db[:, TS(k, P)], identb[:])
            v.tensor_copy(DTt[:, :, TS(i, P)], ptd[:])
            pd = ps_c.tile([P, CX], F32, tag="pc", name=f"pd{i}")
            have_b = not ln_trivial
            for k in range(CT):
                te.matmul(pd[:], DTt[:, k, TS(i, P)], WT[:, k, :],
                          start=(k == 0),
                          stop=(k == CT - 1) and not have_b)
            if have_b:
                te.matmul(pd[:], ones1b[:], bvec[:], start=False, stop=True)
            sqo = scr.tile([P, C], F32, tag="sqo", name="sqo")
            s.activation(sqo[:], pd[:, 0:C], AF.Square,
                         accum_out=SQ[:, i:i + 1])
            v.tensor_scalar_mul(MU[:, i:i + 1], pd[:, C:CX], 1.0 / C)
            v.tensor_mul(MUSQ[:, i:i + 1], MU[:, i:i + 1], MU[:, i:i + 1])
            v.scalar_tensor_tensor(
                out=VAR[:, i:i + 1], in0=SQ[:, i:i + 1], scalar=1.0 / C,
                in1=MUSQ[:, i:i + 1], op0=mult, op1=subtract)
            s.activation(LV[:, i:i + 1], VAR[:, i:i + 1], AF.Ln, bias=eps[:])
            s.activation(RSTD[:, i:i + 1], LV[:, i:i + 1], AF.Exp, scale=-0.5)
            v.scalar_tensor_tensor(
                out=NB[:, i:i + 1], in0=MU[:, i:i + 1], scalar=-1.0,
                in1=RSTD[:, i:i + 1], op0=mult, op1=mult)
            if ln_trivial and use_prelu:
                s.activation(Y[:, i, :], pd[:, 0:C], AF.Prelu,
                             scale=RSTD[:, i:i + 1], bias=NB[:, i:i + 1],
                             alpha=LEAK)
            elif ln_trivial:
                # CoreSim fallback: Prelu isn't implemented there
                tt = scr.tile([P, C], F32, tag="tt", name="tt")
                s.activation(tt[:], pd[:, 0:C], AF.Identity,
                             scale=RSTD[:, i:i + 1], bias=NB[:, i:i + 1])
                v.scalar_tensor_tensor(
                    out=Y[:, i, :], in0=tt[:], scalar=LEAK, in1=tt[:],
                    op0=mult, op1=amax)
            else:
                tt = scr.tile([P, C], F32, tag="tt", name="tt")
                s.activation(tt[:], pd[:, 0:C], AF.Identity,
                             scale=RSTD[:, i:i + 1], bias=NB[:, i:i + 1])
                u = scr.tile([P, C], F32, tag="u", name="u")
                v.tensor_mul(u[:], tt[:], g_bc[:])
                w_ = scr.tile([P, C], F32, tag="w_", name="w_")
                v.tensor_add(w_[:], u[:], be_bc[:])
                v.scalar_tensor_tensor(
                    out=Y[:, i, :], in0=w_[:], scalar=LEAK, in1=w_[:],
                    op0=mult, op1=amax)
            sy.dma_start(y_g[:, i, :], Y[:, i, :])


_PROGRAM_CACHE = {}
last_results = None


def _get_program(diag_one=True, ln_trivial=True):
    key = (diag_one, ln_trivial,
           os.environ.get("KERNEL_WARMUP", "48"),
           os.environ.get("KERNEL_WARMUP_A", "2"),
           os.environ.get("KERNEL_PRELU", "1"))
    if key not in _PROGRAM_CACHE:
        _PROGRAM_CACHE[key] = _build_program(diag_one, ln_trivial)
    return _PROGRAM_CACHE[key]


def _prep_inputs(local_feat, W_adj, W_aff, b_aff, ln_gamma, ln_beta):
    x = np.asarray(local_feat, np.float32).reshape(B, N, C)
    xb = x.astype(BF)
    Wf = np.asarray(W_aff, np.float32)
    wneg = np.ascontiguousarray(-Wf.T).astype(BF)        # [cin, cout]
    w1 = wneg.astype(np.float32).sum(axis=1).astype(BF)  # rowsum column
    wt = np.concatenate([wneg, w1[:, None]], axis=1)     # [C, 257] bf16
    identb = np.eye(P, dtype=np.float32).astype(BF)
    diag = np.ascontiguousarray(np.diagonal(np.asarray(W_adj, np.float32)))
    wd = np.ascontiguousarray(diag.reshape(CT, P).T).astype(np.float32)
    b = np.asarray(b_aff, np.float32).ravel()
    g = np.asarray(ln_gamma, np.float32).ravel()
    be = np.asarray(ln_beta, np.float32).ravel()
    rows = np.concatenate([g, be]).reshape(1, 2 * C).astype(np.float32)
    bvec = np.concatenate([b, [b.sum()]]).reshape(1, CX).astype(BF)
    diag_one = bool(np.all(diag == 1.0))
    ln_trivial = bool(np.all(g == 1.0) and np.all(be == 0.0)
                      and np.all(b == 0.0))
    in_maps = [
        {"xb": np.ascontiguousarray(xb[bb]), "wt": wt, "identb": identb,
         "wdiag": wd, "rows": rows, "bvec": bvec}
        for bb in range(B)
    ]
    return in_maps, diag_one, ln_trivial


def kernel(local_feat, global_feat, pos, W_adj, W_aff, b_aff, ln_gamma,
           ln_beta, **_unused):
    global last_results
    in_maps, diag_one, ln_trivial = _prep_inputs(
        local_feat, W_adj, W_aff, b_aff, ln_gamma, ln_beta)
    nc = _get_program(diag_one, ln_trivial)
    trace = bool(int(os.environ.get("KERNEL_TRACE", "0")))
    res = run_bass_kernel_spmd(nc, in_maps, list(range(B)), trace=trace)
    last_results = res
    out = np.stack([np.asarray(res.results[bb]["y"]) for bb in range(B)],
                   axis=0)
    return out.reshape(B, T, NN, C).astype(np.float32)


# revision 8
# speedup vs baseline: 1.1635x; 1.1635x over previous
"""Trainium2 Bass kernel for nn_Diff_prop_18425409699925 (GNN message passing).

Math (per batch element b, with x = local_feat[b] reshaped to [n=1024, c=256]):
  xn   = x / ||x||_row
  A    = (xn * diag(W_adj)) @ xn^T                (symmetric; einsum uses only
                                                   the diagonal of W_adj)
  G    = exp(5*A) with diagonal zeroed            (the reference's row-max
                                                   shift cancels exactly in the
                                                   row-normalized mean)
  M    = (G @ x) / rowsum(G)
  diff = (x - M) @ W_aff^T + b_aff
  y    = LeakyReLU(LayerNorm(diff) * gamma + beta, 0.01)

Sharding: data-parallel over batch B=8 -> one batch element per NeuronCore,
weights replicated, no collectives. G (symmetric) is used directly as the
lhsT of the G @ x matmul, avoiding a [1024,1024] transpose.

All matmuls and big elementwise ops run in bf16 (operands; PSUM accumulation
is fp32).  Every ScalarE activation in the kernel (Exp, Ln, Square, Copy,
Prelu) lives in the single `natural_log_exp_and_others` table set, so the
~2.7us ACT table load is paid exactly once; sqrt/rsqrt/reciprocal are
computed as exp(-k*ln(s)).

The row-sum of diff (needed for the LayerNorm mean) is obtained for free by
appending the column-sums of W as an extra matmul column.  The sign of W is
flipped host-side so the on-device D' = M - x (natural STT operand order)
yields diff = (x - M) @ W^T.

global_feat and pos are unused by the reference; accepted and ignored.
"""

import os
import sys

import numpy as np

for _p in ("/opt/trn_rl_repo",):
    if os.path.isdir(_p) and _p not in sys.path:
        sys.path.insert(0, _p)

import ml_dtypes
import concourse.bacc as bacc
import concourse.bass as bass
import concourse.tile as tile
from concourse import mybir
from concourse.bass_utils import run_bass_kernel_spmd

B, T, NN, C = 8, 16, 64, 256
N = T * NN            # 1024 nodes per batch element
P = 128               # partitions
NT = N // P           # 8 n-tiles
CT = C // P           # 2 c-tiles
CX = C + 1            # D@W output incl. the rowsum column
F32 = mybir.dt.float32
BF16 = mybir.dt.bfloat16
TS = bass.ts
BF = ml_dtypes.bfloat16

LN_EPS = 1e-5
LEAK = 0.01
DIAG_NEG = -200.0     # added to diagonal of A pre-exp -> exp underflows to 0


def _steered_act_tables(orig_fn):
    """Steer the ACT table-set chooser: Exp and Ln both live in
    `natural_log_exp_and_others`, but the greedy chooser maps each function
    to the FIRST set containing it (natural_log for Ln, exp_and_others for
    Exp), thrashing a ~2.7us table load on every Ln<->Exp alternation.
    Hiding Exp/Ln from every other set makes the chooser settle on the one
    set that contains all activations this kernel uses -> exactly one load.
    Set indices (and hence the emitted act_func_set_id) are unchanged."""
    def wrapped(arch):
        t = orig_fn(arch)
        AF = mybir.ActivationFunctionType
        if "natural_log_exp_and_others" in t:
            for nm in t:
                if nm != "natural_log_exp_and_others":
                    t[nm] = t[nm] - {AF.Exp, AF.Ln}
        return t
    return wrapped


def _build_program(diag_one, ln_trivial):
    nc = bacc.Bacc("TRN2", target_bir_lowering=False, debug=False)
    _orig_gat = bacc.get_activation_tables
    bacc.get_activation_tables = _steered_act_tables(_orig_gat)
    try:
        return _build_program_inner(nc, diag_one, ln_trivial)
    finally:
        bacc.get_activation_tables = _orig_gat


def _build_program_inner(nc, diag_one, ln_trivial):

    xb_d = nc.declare_dram_parameter("xb", [N, C], BF16, isOutput=False)
    wt_d = nc.declare_dram_parameter("wt", [C, CX], BF16, isOutput=False)
    identb_d = nc.declare_dram_parameter("identb", [P, P], BF16, isOutput=False)
    wdiag_d = nc.declare_dram_parameter("wdiag", [P, CT], F32, isOutput=False)
    rows_d = nc.declare_dram_parameter("rows", [1, 2 * C], F32, isOutput=False)
    bvec_d = nc.declare_dram_parameter("bvec", [1, CX], BF16, isOutput=False)
    y_d = nc.declare_dram_parameter("y", [N, C], F32, isOutput=True)

    with tile.TileContext(nc) as tc:
        _emit(nc, tc, xb_d, wt_d, identb_d, wdiag_d, rows_d, bvec_d, y_d,
              diag_one, ln_trivial)
    nc.finalize()
    return nc


def _emit(nc, tc, xb_d, wt_d, identb_d, wdiag_d, rows_d, bvec_d, y_d,
          diag_one, ln_trivial):
    from contextlib import ExitStack

    mult = mybir.AluOpType.mult
    add = mybir.AluOpType.add
    subtract = mybir.AluOpType.subtract
    bypass = mybir.AluOpType.bypass
    amax = mybir.AluOpType.max
    AF = mybir.ActivationFunctionType

    W0 = int(os.environ.get("KERNEL_WARMUP", "48"))
    WA = int(os.environ.get("KERNEL_WARMUP_A", "2"))
    WC = int(os.environ.get("KERNEL_WARMUP_C", "16"))
    use_prelu = bool(int(os.environ.get("KERNEL_PRELU", "1")))

    v = nc.vector
    s = nc.scalar
    te = nc.tensor
    sy = nc.sync

    with ExitStack() as ctx:
        sb = ctx.enter_context(tc.tile_pool(name="sb", bufs=1))
        scr = ctx.enter_context(tc.tile_pool(name="scr", bufs=3))
        ps_a = ctx.enter_context(tc.tile_pool(name="ps_a", bufs=3, space="PSUM"))
        ps_c = ctx.enter_context(tc.tile_pool(name="ps_c", bufs=4, space="PSUM"))
        ps_t = ctx.enter_context(tc.tile_pool(name="ps_t", bufs=1, space="PSUM"))

        # ---------------- persistent SBUF tiles ----------------
        Xb = sb.tile([P, NT, C], BF16, tag="Xb", name="Xb")
        xnT = sb.tile([P, CT, N], BF16, tag="xnT", name="xnT")
        if diag_one:
            xnTs = xnT
        else:
            xnTs = sb.tile([P, CT, N], BF16, tag="xnTs", name="xnTs")
        G = [sb.tile([P, N], BF16, tag=f"G{i}", name=f"G{i}") for i in range(NT)]
        DTt = sb.tile([P, CT, N], BF16, tag="DTt", name="DTt")
        Y = sb.tile([P, NT, C], F32, tag="Y", name="Y")
        WT = sb.tile([P, CT, CX], BF16, tag="WT", name="WT")
        identb = sb.tile([P, P], BF16, tag="identb", name="identb")
        negeye = sb.tile([P, P], F32, tag="negeye", name="negeye")
        warm_src = sb.tile([P, C], BF16, tag="warm_src", name="warm_src")
        eps = sb.tile([P, 1], F32, tag="eps", name="eps")

        # batched per-row stats, one column per n-tile
        def stat(nm):
            return sb.tile([P, NT], F32, tag=nm, name=nm)
        SS, LNS, RNO = stat("SS"), stat("LNS"), stat("RNO")
        SPa, SPb, SP = stat("SPa"), stat("SPb"), stat("SP")
        LSP, NRS = stat("LSP"), stat("NRS")
        SQ, MU, MUSQ = stat("SQ"), stat("MU"), stat("MUSQ")
        VAR, LV, RSTD, NB = stat("VAR"), stat("LV"), stat("RSTD"), stat("NB")

        if not diag_one:
            wdiag = sb.tile([P, CT], F32, tag="wdiag", name="wdiag")
        if not ln_trivial:
            g_bc = sb.tile([P, C], F32, tag="g_bc", name="g_bc")
            be_bc = sb.tile([P, C], F32, tag="be_bc", name="be_bc")
            rows = sb.tile([1, 2 * C], F32, tag="rows", name="rows")
            bvec = sb.tile([1, CX], BF16, tag="bvec", name="bvec")
            ones1f = sb.tile([1, P], F32, tag="ones1f", name="ones1f")
            ones1b = sb.tile([1, P], BF16, tag="ones1b", name="ones1b")

        # ---------------- loads (sync/HWDGE queues) ----------------
        sy.dma_start(identb[:], identb_d[:])
        wt_g = wt_d[:].rearrange("(k p) x -> p k x", p=P)
        sy.dma_start(WT[:], wt_g)
        if not diag_one:
            sy.dma_start(wdiag[:], wdiag_d[:])
        if not ln_trivial:
            sy.dma_start(rows[:], rows_d[:])
            sy.dma_start(bvec[:], bvec_d[:])
        xg = xb_d[:].rearrange("(i p) c -> p i c", p=P)
        for i in range(NT):
            sy.dma_start(Xb[:, i, :], xg[:, i, :])

        v.memset(eps[:], LN_EPS)
        v.memset(warm_src[:], 0.0)
        v.tensor_scalar_mul(negeye[:], identb[:], DIAG_NEG)

        if not ln_trivial:
            v.memset(ones1f[:], 1.0)
            v.memset(ones1b[:], 1.0)
            pg = ps_a.tile([P, 512], F32, tag="pa", name="pg")
            nc.tensor.matmul(pg[:], ones1f[:], rows[:],
                             start=True, stop=True)
            v.tensor_copy(g_bc[:], pg[:, 0:C])
            v.tensor_copy(be_bc[:], pg[:, C:2 * C])

        # PE warm-up: keep the HAM clock gate open during the input-DMA
        # ramp so the matmul phases run at 2.4 GHz from the start.
        pw = None

        def warm(n):
            nonlocal pw
            if pw is None:
                pw = ps_a.tile([P, 512], F32, tag="pa", name="pw")
            for _ in range(n):
                te.matmul(pw[:, 0:C], warm_src[:, 0:P], warm_src[:],
                          start=True, stop=True)

        warm(W0)

        # ---------------- phase A: row-normalize, build xn^T ----------------
        # software-pipelined: rsqrt of tile ii overlaps xn/transpose of ii-1
        for ii in range(NT + 1):
            if ii < NT:
                i = ii
                sqs = scr.tile([P, C], BF16, tag="sqs", name="sqs")
                v.scalar_tensor_tensor(
                    out=sqs[:], in0=Xb[:, i, :], scalar=1.0, in1=Xb[:, i, :],
                    op0=bypass, op1=mult, accum_out=SS[:, i:i + 1])
                s.activation(LNS[:, i:i + 1], SS[:, i:i + 1], AF.Ln)
                s.activation(RNO[:, i:i + 1], LNS[:, i:i + 1], AF.Exp,
                             scale=-0.5)
            if ii >= 1:
                i = ii - 1
                xn = scr.tile([P, C], BF16, tag="xn", name="xn")
                v.tensor_scalar_mul(xn[:], Xb[:, i, :], RNO[:, i:i + 1])
                pt = ps_t.tile([P, CT, P], BF16, tag="pt", name="pt")
                for k in range(CT):
                    te.transpose(pt[:, k, :], xn[:, TS(k, P)], identb[:])
                warm(WA)
                dst = xnT[:, :, TS(i, P)]
                if i % 2 == 0:
                    s.activation(dst, pt[:], AF.Copy)
                else:
                    v.tensor_copy(dst, pt[:])
                if not diag_one:
                    for k in range(CT):
                        v.tensor_scalar_mul(
                            xnTs[:, k, TS(i, P)], pt[:, k, :],
                            wdiag[:, k:k + 1])

        # ---------------- phase B: A = xnTs^T @ xnT, G = exp(5A) ----------------
        # pa is a [P,512] half-row window; exp runs per half so PSUM stays
        # within 3 banks while staying double-buffered.
        for i in range(NT):
            jd, cd = divmod(i, 4)  # diag block lives in window jd, offset cd
            for j in range(2):
                paw = ps_a.tile([P, 512], F32, tag="pa", name=f"pa{i}_{j}")
                for k in range(CT):
                    te.matmul(
                        paw[:],
                        xnTs[:, k, TS(i, P)],
                        xnT[:, k, TS(j, 512)],
                        start=(k == 0), stop=(k == CT - 1))
                if j == jd:
                    v.tensor_add(paw[:, TS(cd, P)], paw[:, TS(cd, P)],
                                 negeye[:])
                sph = SPa if j == 0 else SPb
                s.activation(G[i][:, TS(j, 512)], paw[:], AF.Exp, scale=5.0,
                             accum_out=sph[:, i:i + 1])
        v.tensor_add(SP[:], SPa[:], SPb[:])
        s.activation(LSP[:], SP[:], AF.Ln)
        s.activation(NRS[:], LSP[:], AF.Exp, scale=-1.0)

        # bridge the PE idle gap while the last exps drain, so the HAM
        # clock gate stays open into phase C
        warm(WC)

        # ---------------- phase C: M, diff = (x-M)W^T, LN, LeakyReLU ----------------
        # 3-stage software pipeline: S1 = G@x + D', S2 = transpose + D@W +
        # Square, S3 = LN stats + finisher + store.  Each engine queue sees
        # ops in data-readiness order, so no FIFO head-blocking.
        y_g = y_d[:].rearrange("(i p) c -> p i c", p=P)
        pys = [None] * NT
        pds = [None] * NT
        for ii in range(NT + 2):
            if ii < NT:
                i = ii
                py = ps_c.tile([P, CX], F32, tag="pc", name=f"py{i}")
                pys[i] = py
                for k in range(NT):
                    te.matmul(py[:, 0:C], G[k][:, TS(i, P)], Xb[:, k, :],
                              start=(k == 0), stop=(k == NT - 1))
                # D' = M - x  (sign of W is pre-flipped so diff = (x-M)W^T)
                Db = scr.tile([P, C], BF16, tag="db", name=f"db{i}")
                v.scalar_tensor_tensor(
                    out=Db[:], in0=py[:, 0:C], scalar=NRS[:, i:i + 1],
                    in1=Xb[:, i, :], op0=mult, op1=subtract)
                pys[i] = Db  # keep alive
            if 1 <= ii <= NT:
                i = ii - 1
                Db = pys[i]
                ptd = ps_t.tile([P, CT, P], BF16, tag="pt", name=f"ptd{i}")
                for k in range(CT):
                    te.transpose(ptd[:, k, :], Db[:, TS(k, P)], identb[:])
                v.tensor_copy(DTt[:, :, TS(i, P)], ptd[:])
                pd = ps_c.tile([P, CX], F32, tag="pc", name=f"pd{i}")
                pds[i] = pd
                have_b = not ln_trivial
                for k in range(CT):
                    te.matmul(pd[:], DTt[:, k, TS(i, P)], WT[:, k, :],
                              start=(k == 0),
                              stop=(k == CT - 1) and not have_b)
                if have_b:
                    te.matmul(pd[:], ones1b[:], bvec[:], start=False,
                              stop=True)
                sqo = scr.tile([P, C], F32, tag="sqo", name="sqo")
                s.activation(sqo[:], pd[:, 0:C], AF.Square,
                             accum_out=SQ[:, i:i + 1])
                v.tensor_scalar_mul(MU[:, i:i + 1], pd[:, C:CX], 1.0 / C)
                v.tensor_mul(MUSQ[:, i:i + 1], MU[:, i:i + 1],
                             MU[:, i:i + 1])
                v.scalar_tensor_tensor(
                    out=VAR[:, i:i + 1], in0=SQ[:, i:i + 1], scalar=1.0 / C,
                    in1=MUSQ[:, i:i + 1], op0=mult, op1=subtract)
            if ii >= 2:
                i = ii - 2
                pd = pds[i]
                s.activation(LV[:, i:i + 1], VAR[:, i:i + 1], AF.Ln,
                             bias=eps[:])
                s.activation(RSTD[:, i:i + 1], LV[:, i:i + 1], AF.Exp,
                             scale=-0.5)
                v.scalar_tensor_tensor(
                    out=NB[:, i:i + 1], in0=MU[:, i:i + 1], scalar=-1.0,
                    in1=RSTD[:, i:i + 1], op0=mult, op1=mult)
                if ln_trivial and use_prelu:
                    s.activation(Y[:, i, :], pd[:, 0:C], AF.Prelu,
                                 scale=RSTD[:, i:i + 1], bias=NB[:, i:i + 1],
                                 alpha=LEAK)
                elif ln_trivial:
                    # CoreSim fallback: Prelu isn't implemented there
                    tt = scr.tile([P, C], F32, tag="tt", name="tt")
                    s.activation(tt[:], pd[:, 0:C], AF.Identity,
                                 scale=RSTD[:, i:i + 1], bias=NB[:, i:i + 1])
                    v.scalar_tensor_tensor(
                        out=Y[:, i, :], in0=tt[:], scalar=LEAK, in1=tt[:],
                        op0=mult, op1=amax)
                else:
                    tt = scr.tile([P, C], F32, tag="tt", name="tt")
                    s.activation(tt[:], pd[:, 0:C], AF.Identity,
                                 scale=RSTD[:, i:i + 1], bias=NB[:, i:i + 1])
                    u = scr.tile([P, C], F32, tag="u", name="u")
                    v.tensor_mul(u[:], tt[:], g_bc[:])
                    w_ = scr.tile([P, C], F32, tag="w_", name="w_")
                    v.tensor_add(w_[:], u[:], be_bc[:])
                    v.scalar_tensor_tensor(
                        out=Y[:, i, :], in0=w_[:], scalar=LEAK, in1=w_[:],
                        op0=mult, op1=amax)
                sy.dma_start(y_g[:, i, :], Y[:, i, :])


_PROGRAM_CACHE = {}
last_results = None


def _get_program(diag_one=True, ln_trivial=True):
    key = (diag_one, ln_trivial,
           os.environ.get("KERNEL_WARMUP", "48"),
           os.environ.get("KERNEL_WARMUP_A", "2"),
           os.environ.get("KERNEL_WARMUP_C", "16"),
           os.environ.get("KERNEL_PRELU", "1"))
    if key not in _PROGRAM_CACHE:
        _PROGRAM_CACHE[key] = _build_program(diag_one, ln_trivial)
    return _PROGRAM_CACHE[key]


def _prep_inputs(local_feat, W_adj, W_aff, b_aff, ln_gamma, ln_beta):
    x = np.asarray(local_feat, np.float32).reshape(B, N, C)
    xb = x.astype(BF)
    Wf = np.asarray(W_aff, np.float32)
    wneg = np.ascontiguousarray(-Wf.T).astype(BF)        # [cin, cout]
    w1 = wneg.astype(np.float32).sum(axis=1).astype(BF)  # rowsum column
    wt = np.concatenate([wneg, w1[:, None]], axis=1)     # [C, 257] bf16
    identb = np.eye(P, dtype=np.float32).astype(BF)
    diag = np.ascontiguousarray(np.diagonal(np.asarray(W_adj, np.float32)))
    wd = np.ascontiguousarray(diag.reshape(CT, P).T).astype(np.float32)
    b = np.asarray(b_aff, np.float32).ravel()
    g = np.asarray(ln_gamma, np.float32).ravel()
    be = np.asarray(ln_beta, np.float32).ravel()
    rows = np.concatenate([g, be]).reshape(1, 2 * C).astype(np.float32)
    bvec = np.concatenate([b, [b.sum()]]).reshape(1, CX).astype(BF)
    diag_one = bool(np.all(diag == 1.0))
    ln_trivial = bool(np.all(g == 1.0) and np.all(be == 0.0)
                      and np.all(b == 0.0))
    in_maps = [
        {"xb": np.ascontiguousarray(xb[bb]), "wt": wt, "identb": identb,
         "wdiag": wd, "rows": rows, "bvec": bvec}
        for bb in range(B)
    ]
    return in_maps, diag_one, ln_trivial


def kernel(local_feat, global_feat, pos, W_adj, W_aff, b_aff, ln_gamma,
           ln_beta, **_unused):
    global last_results
    in_maps, diag_one, ln_trivial = _prep_inputs(
        local_feat, W_adj, W_aff, b_aff, ln_gamma, ln_beta)
    nc = _get_program(diag_one, ln_trivial)
    trace = bool(int(os.environ.get("KERNEL_TRACE", "0")))
    res = run_bass_kernel_spmd(nc, in_maps, list(range(B)), trace=trace)
    last_results = res
    out = np.stack([np.asarray(res.results[bb]["y"]) for bb in range(B)],
                   axis=0)
    return out.reshape(B, T, NN, C).astype(np.float32)


# revision 12
# speedup vs baseline: 1.2432x; 1.0685x over previous
"""Trainium2 Bass kernel for nn_Diff_prop_18425409699925 (GNN message passing).

Math (per batch element b, with x = local_feat[b] reshaped to [n=1024, c=256]):
  xn   = x / ||x||_row
  A    = (xn * diag(W_adj)) @ xn^T                (symmetric; einsum uses only
                                                   the diagonal of W_adj)
  G    = exp(5*A) with diagonal zeroed            (the reference's row-max
                                                   shift cancels exactly in the
                                                   row-normalized mean)
  M    = (G @ x) / rowsum(G)
  diff = (x - M) @ W_aff^T + b_aff
  y    = LeakyReLU(LayerNorm(diff) * gamma + beta, 0.01)

Sharding: data-parallel over batch B=8 -> one batch element per NeuronCore,
weights replicated, no collectives. G (symmetric) is used directly as the
lhsT of the G @ x matmul, avoiding a [1024,1024] transpose.

All matmuls and big elementwise ops run in bf16 (operands; PSUM accumulation
is fp32).  Every ScalarE activation in the kernel (Exp, Ln, Square, Copy,
Prelu) lives in the single `natural_log_exp_and_others` table set, so the
~2.7us ACT table load is paid exactly once; sqrt/rsqrt/reciprocal are
computed as exp(-k*ln(s)).

The row-sum of diff (needed for the LayerNorm mean) is obtained for free by
appending the column-sums of W as an extra matmul column.  The sign of W is
flipped host-side so the on-device D' = M - x (natural STT operand order)
yields diff = (x - M) @ W^T.

global_feat and pos are unused by the reference; accepted and ignored.
"""

import os
import sys

import numpy as np

for _p in ("/opt/trn_rl_repo",):
    if os.path.isdir(_p) and _p not in sys.path:
        sys.path.insert(0, _p)

import ml_dtypes
import concourse.bacc as bacc
import concourse.bass as bass
import concourse.tile as tile
from concourse import mybir
from concourse.bass_utils import run_bass_kernel_spmd

B, T, NN, C = 8, 16, 64, 256
N = T * NN            # 1024 nodes per batch element
P = 128               # partitions
NT = N // P           # 8 n-tiles
CT = C // P           # 2 c-tiles
CX = C + 1            # D@W output incl. the rowsum column
F32 = mybir.dt.float32
BF16 = mybir.dt.bfloat16
TS = bass.ts
BF = ml_dtypes.bfloat16

LN_EPS = 1e-5
LEAK = 0.01
DIAG_NEG = -200.0     # added to diagonal of A pre-exp -> exp underflows to 0


def _steered_act_tables(orig_fn):
    """Steer the ACT table-set chooser: Exp and Ln both live in
    `natural_log_exp_and_others`, but the greedy chooser maps each function
    to the FIRST set containing it (natural_log for Ln, exp_and_others for
    Exp), thrashing a ~2.7us table load on every Ln<->Exp alternation.
    Hiding Exp/Ln from every other set makes the chooser settle on the one
    set that contains all activations this kernel uses -> exactly one load.
    Set indices (and hence the emitted act_func_set_id) are unchanged."""
    def wrapped(arch):
        t = orig_fn(arch)
        AF = mybir.ActivationFunctionType
        if "natural_log_exp_and_others" in t:
            for nm in t:
                if nm != "natural_log_exp_and_others":
                    t[nm] = t[nm] - {AF.Exp, AF.Ln}
        return t
    return wrapped


def _build_program(diag_one, ln_trivial):
    nc = bacc.Bacc("TRN2", target_bir_lowering=False, debug=False)
    _orig_gat = bacc.get_activation_tables
    bacc.get_activation_tables = _steered_act_tables(_orig_gat)
    try:
        return _build_program_inner(nc, diag_one, ln_trivial)
    finally:
        bacc.get_activation_tables = _orig_gat


def _build_program_inner(nc, diag_one, ln_trivial):

    xb_d = nc.declare_dram_parameter("xb", [N, C], BF16, isOutput=False)
    wt_d = nc.declare_dram_parameter("wt", [C, CX], BF16, isOutput=False)
    identb_d = nc.declare_dram_parameter("identb", [P, P], BF16, isOutput=False)
    wdiag_d = nc.declare_dram_parameter("wdiag", [P, CT], F32, isOutput=False)
    rows_d = nc.declare_dram_parameter("rows", [1, 2 * C], F32, isOutput=False)
    bvec_d = nc.declare_dram_parameter("bvec", [1, CX], BF16, isOutput=False)
    y_d = nc.declare_dram_parameter("y", [N, C], F32, isOutput=True)

    with tile.TileContext(nc) as tc:
        _emit(nc, tc, xb_d, wt_d, identb_d, wdiag_d, rows_d, bvec_d, y_d,
              diag_one, ln_trivial)
    nc.finalize()
    return nc


def _emit(nc, tc, xb_d, wt_d, identb_d, wdiag_d, rows_d, bvec_d, y_d,
          diag_one, ln_trivial):
    from contextlib import ExitStack

    mult = mybir.AluOpType.mult
    add = mybir.AluOpType.add
    subtract = mybir.AluOpType.subtract
    bypass = mybir.AluOpType.bypass
    amax = mybir.AluOpType.max
    lshr = mybir.AluOpType.logical_shift_right
    bnot = mybir.AluOpType.bitwise_not
    AF = mybir.ActivationFunctionType
    U32 = mybir.dt.uint32

    W0 = int(os.environ.get("KERNEL_WARMUP", "48"))
    WA = int(os.environ.get("KERNEL_WARMUP_A", "2"))
    WB = int(os.environ.get("KERNEL_WARMUP_B", "2"))
    WC = int(os.environ.get("KERNEL_WARMUP_C", "12"))
    use_prelu = bool(int(os.environ.get("KERNEL_PRELU", "1")))

    v = nc.vector
    s = nc.scalar
    te = nc.tensor
    sy = nc.sync
    gp = nc.gpsimd
    dma_eng = [sy, nc.scalar, gp]

    with ExitStack() as ctx:
        sb = ctx.enter_context(tc.tile_pool(name="sb", bufs=1))
        scr = ctx.enter_context(tc.tile_pool(name="scr", bufs=3))
        ps_a = ctx.enter_context(tc.tile_pool(name="ps_a", bufs=2, space="PSUM"))
        ps_c = ctx.enter_context(tc.tile_pool(name="ps_c", bufs=3, space="PSUM"))
        ps_t = ctx.enter_context(tc.tile_pool(name="ps_t", bufs=1, space="PSUM"))

        # ---------------- persistent SBUF tiles ----------------
        # Xb carries a trailing ones column so the G @ x matmul also yields
        # rowsum(G) (the L1 normalizer) in column 256 -- no reduction pass.
        Xb = sb.tile([P, NT, CX], BF16, tag="Xb", name="Xb")
        xnT = sb.tile([P, CT, N], BF16, tag="xnT", name="xnT")
        if diag_one:
            xnTs = xnT
        else:
            xnTs = sb.tile([P, CT, N], BF16, tag="xnTs", name="xnTs")
        G = [sb.tile([P, N], BF16, tag=f"G{i}", name=f"G{i}") for i in range(NT)]
        DTt = sb.tile([P, CT, N], BF16, tag="DTt", name="DTt")
        Y = sb.tile([P, NT, C], F32, tag="Y", name="Y")
        WT = sb.tile([P, CT, CX], BF16, tag="WT", name="WT")
        identb = sb.tile([P, P], BF16, tag="identb", name="identb")
        negeyeb = sb.tile([P, P], BF16, tag="negeyeb", name="negeyeb")
        warm_src = sb.tile([P, C], BF16, tag="warm_src", name="warm_src")

        # batched per-row stats, one column per n-tile
        def stat(nm):
            return sb.tile([P, NT], F32, tag=nm, name=nm)
        SS, LNS, RNO = stat("SS"), stat("LNS"), stat("RNO")
        NRS = stat("NRS")
        SQ, MUn, MUSQ = stat("SQ"), stat("MUn"), stat("MUSQ")
        SQA, VAR, LV, RSTD, NB = (stat("SQA"), stat("VAR"), stat("LV"),
                                  stat("RSTD"), stat("NB"))

        if not diag_one:
            wdiag = sb.tile([P, CT], F32, tag="wdiag", name="wdiag")
        if not ln_trivial:
            g_bc = sb.tile([P, C], F32, tag="g_bc", name="g_bc")
            be_bc = sb.tile([P, C], F32, tag="be_bc", name="be_bc")
            rows = sb.tile([1, 2 * C], F32, tag="rows", name="rows")
            bvec = sb.tile([1, CX], BF16, tag="bvec", name="bvec")
            ones1f = sb.tile([1, P], F32, tag="ones1f", name="ones1f")
            ones1b = sb.tile([1, P], BF16, tag="ones1b", name="ones1b")

        # ---------------- loads (spread across DGE queues) ----------------
        sy.dma_start(identb[:], identb_d[:])
        wt_g = wt_d[:].rearrange("(k p) x -> p k x", p=P)
        gp.dma_start(WT[:], wt_g)
        if not diag_one:
            gp.dma_start(wdiag[:], wdiag_d[:])
        if not ln_trivial:
            gp.dma_start(rows[:], rows_d[:])
            gp.dma_start(bvec[:], bvec_d[:])
        xg = xb_d[:].rearrange("(i p) c -> p i c", p=P)
        for i in range(NT):
            dma_eng[i % 3].dma_start(Xb[:, i, 0:C], xg[:, i, :])

        v.memset(Xb[:, :, C:CX], 1.0)
        v.memset(warm_src[:], 0.0)
        v.tensor_scalar_mul(negeyeb[:], identb[:], DIAG_NEG)

        if not ln_trivial:
            v.memset(ones1f[:], 1.0)
            v.memset(ones1b[:], 1.0)
            pg = ps_a.tile([P, N], F32, tag="pa", name="pg")
            nc.tensor.matmul(pg[:, 0:2 * C], ones1f[:], rows[:],
                             start=True, stop=True)
            v.tensor_copy(g_bc[:], pg[:, 0:C])
            v.tensor_copy(be_bc[:], pg[:, C:2 * C])

        # PE warm-up: keep the HAM clock gate open while the PE would
        # otherwise idle (input-DMA ramp, exp drain), so matmuls run at
        # 2.4 GHz instead of the cold 1.2 GHz.
        pw = None

        def warm(n):
            nonlocal pw
            if pw is None:
                pw = ps_a.tile([P, N], F32, tag="pa", name="pw")
            for _ in range(n):
                te.matmul(pw[:, 0:C], warm_src[:, 0:P], warm_src[:],
                          start=True, stop=True)

        warm(W0)

        # ---------------- phase A: row-normalize, build xn^T ----------------
        for i in range(NT):
            sqs = scr.tile([P, C], BF16, tag="sqs", name="sqs")
            v.scalar_tensor_tensor(
                out=sqs[:], in0=Xb[:, i, 0:C], scalar=1.0, in1=Xb[:, i, 0:C],
                op0=bypass, op1=mult, accum_out=SS[:, i:i + 1])
        s.activation(LNS[:], SS[:], AF.Ln)
        s.activation(RNO[:], LNS[:], AF.Exp, scale=-0.5)
        for i in range(NT):
            xn = scr.tile([P, C], BF16, tag="xn", name="xn")
            v.tensor_scalar_mul(xn[:], Xb[:, i, 0:C], RNO[:, i:i + 1])
            pt = ps_t.tile([P, CT, P], BF16, tag="pt", name="pt")
            for k in range(CT):
                te.transpose(pt[:, k, :], xn[:, TS(k, P)], identb[:])
            warm(WA)
            dst = xnT[:, :, TS(i, P)]
            if i % 2 == 0:
                s.activation(dst, pt[:], AF.Copy)
            else:
                v.tensor_copy(dst, pt[:])
            if not diag_one:
                for k in range(CT):
                    v.tensor_scalar_mul(
                        xnTs[:, k, TS(i, P)], pt[:, k, :], wdiag[:, k:k + 1])

        # ---------------- phase B: A = xnTs^T @ xnT, G = exp(5A) ----------------
        # The -200*I diagonal knockout is an extra accumulating matmul, so
        # exp depends only on the PE and ScalarE streams exp back-to-back.
        for i in range(NT):
            jd = i // 4
            pa = ps_a.tile([P, N], F32, tag="pa", name=f"pa{i}")
            for j in range(2):
                for k in range(CT):
                    te.matmul(
                        pa[:, TS(j, 512)],
                        xnTs[:, k, TS(i, P)],
                        xnT[:, k, TS(j, 512)],
                        start=(k == 0),
                        stop=(k == CT - 1) and (j != jd))
            te.matmul(pa[:, TS(i, P)], negeyeb[:], identb[:],
                      start=False, stop=True)
            warm(WB)
            s.activation(G[i][:], pa[:], AF.Exp, scale=5.0)
        warm(WC)

        # ---------------- phase C: M, diff = (x-M)W^T, LN, LeakyReLU ----------------
        # 3-stage software pipeline: S1 = G@x + D', S2 = transpose + D@W +
        # Square + unnormalized Prelu, S3 (per 4-tile group) = batched
        # rstd + scale + store.  LeakyReLU is positively homogeneous, so
        # Prelu(pd - mu) happens while pd is hot in PSUM and the 1/std
        # scaling moves to a cheap DVE tensor_scalar afterward.
        y_g = y_d[:].rearrange("(i p) c -> p i c", p=P)
        Dbs = [None] * NT
        qs = [None] * NT
        qpool = ctx.enter_context(tc.tile_pool(name="qpool", bufs=5))

        def finish_group(g):
            i0 = 4 * g
            s.activation(LV[:, i0:i0 + 4], VAR[:, i0:i0 + 4], AF.Ln)
            s.activation(RSTD[:, i0:i0 + 4], LV[:, i0:i0 + 4], AF.Exp,
                         scale=-0.5)
            for i in range(i0, i0 + 4):
                if ln_trivial:
                    v.tensor_scalar_mul(Y[:, i, :], qs[i], RSTD[:, i:i + 1])
                else:
                    # q = diff - mu; t = q*rstd, then gamma/beta + leaky
                    u = scr.tile([P, C], F32, tag="u", name="u")
                    v.tensor_scalar_mul(u[:], qs[i], RSTD[:, i:i + 1])
                    uu = scr.tile([P, C], F32, tag="uu", name="uu")
                    v.tensor_mul(uu[:], u[:], g_bc[:])
                    w_ = scr.tile([P, C], F32, tag="w_", name="w_")
                    v.tensor_add(w_[:], uu[:], be_bc[:])
                    v.scalar_tensor_tensor(
                        out=Y[:, i, :], in0=w_[:], scalar=LEAK, in1=w_[:],
                        op0=mult, op1=amax)
                (sy if i % 2 == 0 else gp).dma_start(y_g[:, i, :], Y[:, i, :])

        for ii in range(NT + 2):
            if ii < NT:
                i = ii
                py = ps_c.tile([P, CX], F32, tag="pc", name=f"py{i}")
                for k in range(NT):
                    te.matmul(py[:], G[k][:, TS(i, P)], Xb[:, k, :],
                              start=(k == 0), stop=(k == NT - 1))
                # py[:,256] = rowsum(G_i); NRS = 1/rowsum
                v.reciprocal(NRS[:, i:i + 1], py[:, C:CX])
                # D' = M - x  (sign of W is pre-flipped so diff = (x-M)W^T)
                Db = scr.tile([P, C], BF16, tag="db", name=f"db{i}")
                v.scalar_tensor_tensor(
                    out=Db[:], in0=py[:, 0:C], scalar=NRS[:, i:i + 1],
                    in1=Xb[:, i, 0:C], op0=mult, op1=subtract)
                Dbs[i] = Db
            if 1 <= ii <= NT:
                i = ii - 1
                Db = Dbs[i]
                ptd = ps_t.tile([P, CT, P], BF16, tag="pt", name=f"ptd{i}")
                for k in range(CT):
                    te.transpose(ptd[:, k, :], Db[:, TS(k, P)], identb[:])
                v.tensor_copy(DTt[:, :, TS(i, P)], ptd[:])
                pd = ps_c.tile([P, CX], F32, tag="pc", name=f"pd{i}")
                have_b = not ln_trivial
                for k in range(CT):
                    te.matmul(pd[:], DTt[:, k, TS(i, P)], WT[:, k, :],
                              start=(k == 0),
                              stop=(k == CT - 1) and not have_b)
                if have_b:
                    te.matmul(pd[:], ones1b[:], bvec[:], start=False,
                              stop=True)
                sqo = scr.tile([P, C], F32, tag="sqo", name="sqo")
                s.activation(sqo[:], pd[:, 0:C], AF.Square,
                             accum_out=SQ[:, i:i + 1])
                v.tensor_scalar_mul(MUn[:, i:i + 1], pd[:, C:CX], -1.0 / C)
                v.tensor_mul(MUSQ[:, i:i + 1], MUn[:, i:i + 1],
                             MUn[:, i:i + 1])
                v.tensor_scalar(
                    out=SQA[:, i:i + 1], in0=SQ[:, i:i + 1],
                    scalar1=1.0 / C, scalar2=LN_EPS, op0=mult, op1=add)
                v.scalar_tensor_tensor(
                    out=VAR[:, i:i + 1], in0=SQA[:, i:i + 1], scalar=1.0,
                    in1=MUSQ[:, i:i + 1], op0=bypass, op1=subtract)
                q = qpool.tile([P, C], F32, tag="q", name=f"q{i}")
                qs[i] = q
                if ln_trivial and use_prelu:
                    s.activation(q[:], pd[:, 0:C], AF.Prelu,
                                 bias=MUn[:, i:i + 1], alpha=LEAK)
                elif ln_trivial:
                    # CoreSim fallback: Prelu isn't implemented there
                    tt = scr.tile([P, C], F32, tag="tt", name="tt")
                    s.activation(tt[:], pd[:, 0:C], AF.Identity,
                                 bias=MUn[:, i:i + 1])
                    v.scalar_tensor_tensor(
                        out=q[:], in0=tt[:], scalar=LEAK, in1=tt[:],
                        op0=mult, op1=amax)
                else:
                    # generic path: keep (diff - mu) unactivated; gamma/beta
                    # and the leaky relu are applied in finish_group
                    s.activation(q[:], pd[:, 0:C], AF.Identity,
                                 bias=MUn[:, i:i + 1])
                if i % 4 == 3:
                    finish_group(i // 4)


_PROGRAM_CACHE = {}
last_results = None


def _get_program(diag_one=True, ln_trivial=True):
    key = (diag_one, ln_trivial,
           os.environ.get("KERNEL_WARMUP", "48"),
           os.environ.get("KERNEL_WARMUP_A", "2"),
           os.environ.get("KERNEL_WARMUP_B", "2"),
           os.environ.get("KERNEL_WARMUP_C", "12"),
           os.environ.get("KERNEL_PRELU", "1"))
    if key not in _PROGRAM_CACHE:
        _PROGRAM_CACHE[key] = _build_program(diag_one, ln_trivial)
    return _PROGRAM_CACHE[key]


def _prep_inputs(local_feat, W_adj, W_aff, b_aff, ln_gamma, ln_beta):
    x = np.asarray(local_feat, np.float32).reshape(B, N, C)
    xb = x.astype(BF)
    Wf = np.asarray(W_aff, np.float32)
    wneg = np.ascontiguousarray(-Wf.T).astype(BF)        # [cin, cout]
    w1 = wneg.astype(np.float32).sum(axis=1).astype(BF)  # rowsum column
    wt = np.concatenate([wneg, w1[:, None]], axis=1)     # [C, 257] bf16
    identb = np.eye(P, dtype=np.float32).astype(BF)
    diag = np.ascontiguousarray(np.diagonal(np.asarray(W_adj, np.float32)))
    wd = np.ascontiguousarray(diag.reshape(CT, P).T).astype(np.float32)
    b = np.asarray(b_aff, np.float32).ravel()
    g = np.asarray(ln_gamma, np.float32).ravel()
    be = np.asarray(ln_beta, np.float32).ravel()
    rows = np.concatenate([g, be]).reshape(1, 2 * C).astype(np.float32)
    bvec = np.concatenate([b, [b.sum()]]).reshape(1, CX).astype(BF)
    diag_one = bool(np.all(diag == 1.0))
    ln_trivial = bool(np.all(g == 1.0) and np.all(be == 0.0)
                      and np.all(b == 0.0))
    in_maps = [
        {"xb": np.ascontiguousarray(xb[bb]), "wt": wt, "identb": identb,
         "wdiag": wd, "rows": rows, "bvec": bvec}
        for bb in range(B)
    ]
    return in_maps, diag_one, ln_trivial


def kernel(local_feat, global_feat, pos, W_adj, W_aff, b_aff, ln_gamma,
           ln_beta, **_unused):
    global last_results
    in_maps, diag_one, ln_trivial = _prep_inputs(
        local_feat, W_adj, W_aff, b_aff, ln_gamma, ln_beta)
    nc = _get_program(diag_one, ln_trivial)
    trace = bool(int(os.environ.get("KERNEL_TRACE", "0")))
    res = run_bass_kernel_spmd(nc, in_maps, list(range(B)), trace=trace)
    last_results = res
    out = np.stack([np.asarray(res.results[bb]["y"]) for bb in range(B)],
                   axis=0)
    return out.reshape(B, T, NN, C).astype(np.float32)


# revision 17
# speedup vs baseline: 1.3375x; 1.0758x over previous
"""Trainium2 Bass kernel for nn_Diff_prop_18425409699925 (GNN message passing).

Math (per batch element b, with x = local_feat[b] reshaped to [n=1024, c=256]):
  xn   = x / ||x||_row
  A    = (xn * diag(W_adj)) @ xn^T                (symmetric; einsum uses only
                                                   the diagonal of W_adj)
  G    = exp(5*A) with diagonal zeroed            (the reference's row-max
                                                   shift cancels exactly in the
                                                   row-normalized mean)
  M    = (G @ x) / rowsum(G)
  diff = (x - M) @ W_aff^T + b_aff
  y    = LeakyReLU(LayerNorm(diff) * gamma + beta, 0.01)

Sharding: data-parallel over batch B=8 -> one batch element per NeuronCore,
weights replicated, no collectives. G (symmetric) is used directly as the
lhsT of the G @ x matmul, avoiding a [1024,1024] transpose.

All matmuls and big elementwise ops run in bf16 (operands; PSUM accumulation
is fp32).  Every ScalarE activation in the kernel (Exp, Ln, Square, Copy,
Prelu) lives in the single `natural_log_exp_and_others` table set, so the
~2.7us ACT table load is paid exactly once; sqrt/rsqrt/reciprocal are
computed as exp(-k*ln(s)).

The row-sum of diff (needed for the LayerNorm mean) is obtained for free by
appending the column-sums of W as an extra matmul column.  The sign of W is
flipped host-side so the on-device D' = M - x (natural STT operand order)
yields diff = (x - M) @ W^T.

global_feat and pos are unused by the reference; accepted and ignored.
"""

import os
import sys

import numpy as np

for _p in ("/opt/trn_rl_repo",):
    if os.path.isdir(_p) and _p not in sys.path:
        sys.path.insert(0, _p)

import ml_dtypes
import concourse.bacc as bacc
import concourse.bass as bass
import concourse.tile as tile
from concourse import mybir
from concourse.bass_utils import run_bass_kernel_spmd

B, T, NN, C = 8, 16, 64, 256
N = T * NN            # 1024 nodes per batch element
P = 128               # partitions
NT = N // P           # 8 n-tiles
CT = C // P           # 2 c-tiles
CX = C + 1            # D@W output incl. the rowsum column
F32 = mybir.dt.float32
BF16 = mybir.dt.bfloat16
TS = bass.ts
BF = ml_dtypes.bfloat16

LN_EPS = 1e-5
LEAK = 0.01
DIAG_NEG = -200.0     # added to diagonal of A pre-exp -> exp underflows to 0


def _steered_act_tables(orig_fn):
    """Steer the ACT table-set chooser: Exp and Ln both live in
    `natural_log_exp_and_others`, but the greedy chooser maps each function
    to the FIRST set containing it (natural_log for Ln, exp_and_others for
    Exp), thrashing a ~2.7us table load on every Ln<->Exp alternation.
    Emptying every other set makes the chooser settle on the one set that
    contains all activations this kernel uses -> exactly one load.  Set
    indices (and hence the emitted act_func_set_id) are unchanged."""
    def wrapped(arch):
        t = orig_fn(arch)
        AF = mybir.ActivationFunctionType
        if "natural_log_exp_and_others" in t:
            for nm in t:
                if nm != "natural_log_exp_and_others":
                    t[nm] = set()
        return t
    return wrapped


def _build_program(diag_one, ln_trivial):
    nc = bacc.Bacc("TRN2", target_bir_lowering=False, debug=False)
    _orig_gat = bacc.get_activation_tables
    bacc.get_activation_tables = _steered_act_tables(_orig_gat)
    try:
        return _build_program_inner(nc, diag_one, ln_trivial)
    finally:
        bacc.get_activation_tables = _orig_gat


def _build_program_inner(nc, diag_one, ln_trivial):

    xb_d = nc.declare_dram_parameter("xb", [P, NT, CX], BF16, isOutput=False)
    wt_d = nc.declare_dram_parameter("wt", [P, CT, CX], BF16, isOutput=False)
    identb_d = nc.declare_dram_parameter("identb", [P, P], BF16, isOutput=False)
    wdiag_d = nc.declare_dram_parameter("wdiag", [P, CT], F32, isOutput=False)
    rows_d = nc.declare_dram_parameter("rows", [1, 2 * C], F32, isOutput=False)
    bvec_d = nc.declare_dram_parameter("bvec", [1, CX], BF16, isOutput=False)
    y_d = nc.declare_dram_parameter("y", [P, NT, C], F32, isOutput=True)

    with tile.TileContext(nc) as tc:
        _emit(nc, tc, xb_d, wt_d, identb_d, wdiag_d, rows_d, bvec_d, y_d,
              diag_one, ln_trivial)
    nc.finalize()
    return nc


def _emit(nc, tc, xb_d, wt_d, identb_d, wdiag_d, rows_d, bvec_d, y_d,
          diag_one, ln_trivial):
    from contextlib import ExitStack

    mult = mybir.AluOpType.mult
    add = mybir.AluOpType.add
    subtract = mybir.AluOpType.subtract
    bypass = mybir.AluOpType.bypass
    amax = mybir.AluOpType.max
    lshr = mybir.AluOpType.logical_shift_right
    bnot = mybir.AluOpType.bitwise_not
    AF = mybir.ActivationFunctionType
    U32 = mybir.dt.uint32

    W0 = int(os.environ.get("KERNEL_WARMUP", "28"))
    WA = int(os.environ.get("KERNEL_WARMUP_A", "1"))
    WB = int(os.environ.get("KERNEL_WARMUP_B", "2"))
    WC = int(os.environ.get("KERNEL_WARMUP_C", "12"))
    use_prelu = bool(int(os.environ.get("KERNEL_PRELU", "1")))

    v = nc.vector
    s = nc.scalar
    te = nc.tensor
    sy = nc.sync
    gp = nc.gpsimd
    dma_eng = [sy, nc.scalar, gp]

    with ExitStack() as ctx:
        sb = ctx.enter_context(tc.tile_pool(name="sb", bufs=1))
        scr = ctx.enter_context(tc.tile_pool(name="scr", bufs=3))
        ps_a = ctx.enter_context(tc.tile_pool(name="ps_a", bufs=2, space="PSUM"))
        ps_c = ctx.enter_context(tc.tile_pool(name="ps_c", bufs=3, space="PSUM"))
        ps_t = ctx.enter_context(tc.tile_pool(name="ps_t", bufs=1, space="PSUM"))

        # ---------------- persistent SBUF tiles ----------------
        # Xb carries a trailing ones column so the G @ x matmul also yields
        # rowsum(G) (the L1 normalizer) in column 256 -- no reduction pass.
        Xb = sb.tile([P, NT, CX], BF16, tag="Xb", name="Xb")
        xnT = sb.tile([P, CT, N], BF16, tag="xnT", name="xnT")
        if diag_one:
            xnTs = xnT
        else:
            xnTs = sb.tile([P, CT, N], BF16, tag="xnTs", name="xnTs")
        G = [sb.tile([P, N], BF16, tag=f"G{i}", name=f"G{i}") for i in range(NT)]
        DTt = sb.tile([P, CT, N], BF16, tag="DTt", name="DTt")
        Y = sb.tile([P, NT, C], F32, tag="Y", name="Y")
        WT = sb.tile([P, CT, CX], BF16, tag="WT", name="WT")
        identb = sb.tile([P, P], BF16, tag="identb", name="identb")
        negeyeb = sb.tile([P, P], BF16, tag="negeyeb", name="negeyeb")
        warm_src = sb.tile([P, C], BF16, tag="warm_src", name="warm_src")

        # batched per-row stats, one column per n-tile
        def stat(nm):
            return sb.tile([P, NT], F32, tag=nm, name=nm)
        SS, LNS, RNO = stat("SS"), stat("LNS"), stat("RNO")
        NRS = stat("NRS")
        SQ, MUn, MUSQ = stat("SQ"), stat("MUn"), stat("MUSQ")
        SQA, VAR, LV, RSTD, NB = (stat("SQA"), stat("VAR"), stat("LV"),
                                  stat("RSTD"), stat("NB"))

        if not diag_one:
            wdiag = sb.tile([P, CT], F32, tag="wdiag", name="wdiag")
        if not ln_trivial:
            g_bc = sb.tile([P, C], F32, tag="g_bc", name="g_bc")
            be_bc = sb.tile([P, C], F32, tag="be_bc", name="be_bc")
            rows = sb.tile([1, 2 * C], F32, tag="rows", name="rows")
            bvec = sb.tile([1, CX], BF16, tag="bvec", name="bvec")
            ones1f = sb.tile([1, P], F32, tag="ones1f", name="ones1f")
            ones1b = sb.tile([1, P], BF16, tag="ones1b", name="ones1b")

        # ---------------- loads (spread across DGE queues) ----------------
        # xb/wt/y are partition-major in DRAM (host handles the reshuffle),
        # so every partition's data is one contiguous descriptor.
        sy.dma_start(identb[:], identb_d[:])
        gp.dma_start(WT[:], wt_d[:])
        if not diag_one:
            gp.dma_start(wdiag[:], wdiag_d[:])
        if not ln_trivial:
            gp.dma_start(rows[:], rows_d[:])
            gp.dma_start(bvec[:], bvec_d[:])
        h = NT // 2
        sy.dma_start(Xb[:, 0:h, :], xb_d[:, 0:h, :])
        nc.scalar.dma_start(Xb[:, h:NT, :], xb_d[:, h:NT, :])

        v.memset(warm_src[:], 0.0)
        v.tensor_scalar_mul(negeyeb[:], identb[:], DIAG_NEG)

        if not ln_trivial:
            v.memset(ones1f[:], 1.0)
            v.memset(ones1b[:], 1.0)
            pg = ps_a.tile([P, N], F32, tag="pa", name="pg")
            nc.tensor.matmul(pg[:, 0:2 * C], ones1f[:], rows[:],
                             start=True, stop=True)
            v.tensor_copy(g_bc[:], pg[:, 0:C])
            v.tensor_copy(be_bc[:], pg[:, C:2 * C])

        # PE warm-up: keep the HAM clock gate open while the PE would
        # otherwise idle (input-DMA ramp, exp drain), so matmuls run at
        # 2.4 GHz instead of the cold 1.2 GHz.
        pw = None

        def warm(n):
            nonlocal pw
            if pw is None:
                pw = ps_a.tile([P, N], F32, tag="pa", name="pw")
            for _ in range(n):
                te.matmul(pw[:, 0:C], warm_src[:, 0:P], warm_src[:],
                          start=True, stop=True)

        warm(W0)

        # ---------------- phase A: row-normalize, build xn^T ----------------
        # two half-batches so tiles 0-3 process while tiles 4-7 still load;
        # rsqrt = exp(-0.5*ln) on ScalarE, batched per half
        for hh in range(2):
            lo, hi = hh * (NT // 2), (hh + 1) * (NT // 2)
            for i in range(lo, hi):
                sqs = scr.tile([P, C], BF16, tag="sqs", name="sqs")
                v.scalar_tensor_tensor(
                    out=sqs[:], in0=Xb[:, i, 0:C], scalar=1.0,
                    in1=Xb[:, i, 0:C],
                    op0=bypass, op1=mult, accum_out=SS[:, i:i + 1])
            s.activation(LNS[:, lo:hi], SS[:, lo:hi], AF.Ln)
            s.activation(RNO[:, lo:hi], LNS[:, lo:hi], AF.Exp, scale=-0.5)
            xns = {}
            for i in range(lo, hi):
                xn = scr.tile([P, C], BF16, tag="xn", name=f"xn{i}",
                              bufs=4)
                v.tensor_scalar_mul(xn[:], Xb[:, i, 0:C], RNO[:, i:i + 1])
                xns[i] = xn
            for i in range(lo, hi):
                pt = ps_t.tile([P, CT, P], BF16, tag="pt", name="pt")
                for k in range(CT):
                    te.transpose(pt[:, k, :], xns[i][:, TS(k, P)], identb[:])
                warm(WA)
                dst = xnT[:, :, TS(i, P)]
                if i % 2 == 0:
                    s.activation(dst, pt[:], AF.Copy)
                else:
                    v.tensor_copy(dst, pt[:])
                if not diag_one:
                    for k in range(CT):
                        v.tensor_scalar_mul(
                            xnTs[:, k, TS(i, P)], pt[:, k, :],
                            wdiag[:, k:k + 1])

        # ---------------- phase B: A = xnTs^T @ xnT, G = exp(5A) ----------------
        # The -200*I diagonal knockout is an extra accumulating matmul, so
        # exp depends only on the PE and ScalarE streams exp back-to-back.
        for i in range(NT):
            jd = i // 4
            pa = ps_a.tile([P, N], F32, tag="pa", name=f"pa{i}")
            for j in range(2):
                for k in range(CT):
                    te.matmul(
                        pa[:, TS(j, 512)],
                        xnTs[:, k, TS(i, P)],
                        xnT[:, k, TS(j, 512)],
                        start=(k == 0),
                        stop=(k == CT - 1) and (j != jd))
            te.matmul(pa[:, TS(i, P)], negeyeb[:], identb[:],
                      start=False, stop=True)
            warm(WB)
            s.activation(G[i][:], pa[:], AF.Exp, scale=5.0)
        warm(WC)

        # ---------------- phase C: M, diff = (x-M)W^T, LN, LeakyReLU ----------------
        # 3-stage software pipeline with a 2-block skew: S1 = G@x + 1/rowsum
        # + D', S2 (two blocks later) = transpose + D@W + Square + Prelu(pd-mu),
        # S3 (per 4-tile group) = batched rstd + scale + store.  LeakyReLU is
        # positively homogeneous, so the 1/std scaling happens after the
        # activation on DVE and pd's PSUM lifetime ends inside S2.  Engine
        # queues are ordered by data readiness (transpose first on PE, copy
        # first on DVE) so nothing head-blocks.
        y_g = y_d
        Dbs = [None] * NT
        qs = [None] * NT
        qpool = ctx.enter_context(tc.tile_pool(name="qpool", bufs=5))

        def finish_group(i0, n):
            s.activation(LV[:, i0:i0 + n], VAR[:, i0:i0 + n], AF.Ln)
            s.activation(RSTD[:, i0:i0 + n], LV[:, i0:i0 + n], AF.Exp,
                         scale=-0.5)
            for i in range(i0, i0 + n):
                if ln_trivial:
                    v.tensor_scalar_mul(Y[:, i, :], qs[i], RSTD[:, i:i + 1])
                else:
                    # q = diff - mu; t = q*rstd, then gamma/beta + leaky
                    u = scr.tile([P, C], F32, tag="u", name="u")
                    v.tensor_scalar_mul(u[:], qs[i], RSTD[:, i:i + 1])
                    uu = scr.tile([P, C], F32, tag="uu", name="uu")
                    v.tensor_mul(uu[:], u[:], g_bc[:])
                    w_ = scr.tile([P, C], F32, tag="w_", name="w_")
                    v.tensor_add(w_[:], uu[:], be_bc[:])
                    v.scalar_tensor_tensor(
                        out=Y[:, i, :], in0=w_[:], scalar=LEAK, in1=w_[:],
                        op0=mult, op1=amax)
                (sy if i % 2 == 0 else nc.scalar).dma_start(
                    y_g[:, i, :], Y[:, i, :])

        for ii in range(NT + 2):
            j = ii - 2  # S2 tile index
            if 0 <= j:
                # PE: transpose first -- Db_j has been ready for a block
                Db = Dbs[j]
                ptd = ps_t.tile([P, CT, P], BF16, tag="pt", name=f"ptd{j}")
                for k in range(CT):
                    te.transpose(ptd[:, k, :], Db[:, TS(k, P)], identb[:])
            if ii < NT:
                i = ii
                py = ps_c.tile([P, CX], F32, tag="pc", name=f"py{i}")
                for k in range(NT):
                    te.matmul(py[:], G[k][:, TS(i, P)], Xb[:, k, :],
                              start=(k == 0), stop=(k == NT - 1))
            if 0 <= j:
                v.tensor_copy(DTt[:, :, TS(j, P)], ptd[:])
                pd = ps_c.tile([P, CX], F32, tag="pc", name=f"pd{j}")
                have_b = not ln_trivial
                for k in range(CT):
                    te.matmul(pd[:], DTt[:, k, TS(j, P)], WT[:, k, :],
                              start=(k == 0),
                              stop=(k == CT - 1) and not have_b)
                if have_b:
                    te.matmul(pd[:], ones1b[:], bvec[:], start=False,
                              stop=True)
                sqo = scr.tile([P, C], F32, tag="sqo", name="sqo")
                s.activation(sqo[:], pd[:, 0:C], AF.Square,
                             accum_out=SQ[:, j:j + 1])
            if ii < NT:
                i = ii
                # py[:,256] = rowsum(G_i); NRS = 1/rowsum
                v.reciprocal(NRS[:, i:i + 1], py[:, C:CX])
            if 0 <= j:
                v.tensor_scalar_mul(MUn[:, j:j + 1], pd[:, C:CX], -1.0 / C)
                v.tensor_mul(MUSQ[:, j:j + 1], MUn[:, j:j + 1],
                             MUn[:, j:j + 1])
                v.tensor_scalar(
                    out=SQA[:, j:j + 1], in0=SQ[:, j:j + 1],
                    scalar1=1.0 / C, scalar2=LN_EPS, op0=mult, op1=add)
                v.scalar_tensor_tensor(
                    out=VAR[:, j:j + 1], in0=SQA[:, j:j + 1], scalar=1.0,
                    in1=MUSQ[:, j:j + 1], op0=bypass, op1=subtract)
                q = qpool.tile([P, C], F32, tag="q", name=f"q{j}")
                qs[j] = q
                if ln_trivial and use_prelu:
                    s.activation(q[:], pd[:, 0:C], AF.Prelu,
                                 bias=MUn[:, j:j + 1], alpha=LEAK)
                elif ln_trivial:
                    # CoreSim fallback: Prelu isn't implemented there
                    tt = scr.tile([P, C], F32, tag="tt", name="tt")
                    s.activation(tt[:], pd[:, 0:C], AF.Identity,
                                 bias=MUn[:, j:j + 1])
                    v.scalar_tensor_tensor(
                        out=q[:], in0=tt[:], scalar=LEAK, in1=tt[:],
                        op0=mult, op1=amax)
                else:
                    # generic path: keep (diff - mu) unactivated; gamma/beta
                    # and the leaky relu are applied in finish_group
                    s.activation(q[:], pd[:, 0:C], AF.Identity,
                                 bias=MUn[:, j:j + 1])
            if ii < NT:
                i = ii
                # D' = M - x  (sign of W is pre-flipped so diff = (x-M)W^T)
                Db = scr.tile([P, C], BF16, tag="db", name=f"db{i}")
                v.scalar_tensor_tensor(
                    out=Db[:], in0=py[:, 0:C], scalar=NRS[:, i:i + 1],
                    in1=Xb[:, i, 0:C], op0=mult, op1=subtract)
                Dbs[i] = Db
            if j == 3:
                finish_group(0, 4)
            elif j in (5, 7):
                finish_group(j - 1, 2)


_PROGRAM_CACHE = {}
last_results = None


def _get_program(diag_one=True, ln_trivial=True):
    key = (diag_one, ln_trivial,
           os.environ.get("KERNEL_WARMUP", "48"),
           os.environ.get("KERNEL_WARMUP_A", "2"),
           os.environ.get("KERNEL_WARMUP_B", "2"),
           os.environ.get("KERNEL_WARMUP_C", "12"),
           os.environ.get("KERNEL_PRELU", "1"))
    if key not in _PROGRAM_CACHE:
        _PROGRAM_CACHE[key] = _build_program(diag_one, ln_trivial)
    return _PROGRAM_CACHE[key]


def _prep_inputs(local_feat, W_adj, W_aff, b_aff, ln_gamma, ln_beta):
    x = np.asarray(local_feat, np.float32).reshape(B, N, C)
    # partition-major layout with a trailing ones column: xb_r[b, p, i, :] =
    # [x[b, i*128+p, :], 1.0] -- every SBUF partition's data is one
    # contiguous DMA descriptor, and the ones column makes the G @ x matmul
    # also produce rowsum(G).
    xb = np.ones((B, P, NT, CX), dtype=BF)
    xb[:, :, :, 0:C] = x.reshape(B, NT, P, C).transpose(0, 2, 1, 3).astype(BF)
    Wf = np.asarray(W_aff, np.float32)
    wneg = np.ascontiguousarray(-Wf.T).astype(BF)        # [cin, cout]
    w1 = wneg.astype(np.float32).sum(axis=1).astype(BF)  # rowsum column
    wt = np.concatenate([wneg, w1[:, None]], axis=1)     # [C, 257] bf16
    wt_r = np.ascontiguousarray(
        wt.reshape(CT, P, CX).transpose(1, 0, 2))        # [P, CT, 257]
    identb = np.eye(P, dtype=np.float32).astype(BF)
    diag = np.ascontiguousarray(np.diagonal(np.asarray(W_adj, np.float32)))
    wd = np.ascontiguousarray(diag.reshape(CT, P).T).astype(np.float32)
    b = np.asarray(b_aff, np.float32).ravel()
    g = np.asarray(ln_gamma, np.float32).ravel()
    be = np.asarray(ln_beta, np.float32).ravel()
    rows = np.concatenate([g, be]).reshape(1, 2 * C).astype(np.float32)
    bvec = np.concatenate([b, [b.sum()]]).reshape(1, CX).astype(BF)
    diag_one = bool(np.all(diag == 1.0))
    ln_trivial = bool(np.all(g == 1.0) and np.all(be == 0.0)
                      and np.all(b == 0.0))
    in_maps = [
        {"xb": np.ascontiguousarray(xb[bb]), "wt": wt_r, "identb": identb,
         "wdiag": wd, "rows": rows, "bvec": bvec}
        for bb in range(B)
    ]
    return in_maps, diag_one, ln_trivial


def kernel(local_feat, global_feat, pos, W_adj, W_aff, b_aff, ln_gamma,
           ln_beta, **_unused):
    global last_results
    in_maps, diag_one, ln_trivial = _prep_inputs(
        local_feat, W_adj, W_aff, b_aff, ln_gamma, ln_beta)
    nc = _get_program(diag_one, ln_trivial)
    trace = bool(int(os.environ.get("KERNEL_TRACE", "0")))
    res = run_bass_kernel_spmd(nc, in_maps, list(range(B)), trace=trace)
    last_results = res
    out = np.stack([np.asarray(res.results[bb]["y"]) for bb in range(B)],
                   axis=0)                                # [B, P, NT, C]
    out = out.transpose(0, 2, 1, 3).reshape(B, N, C)      # n = i*128 + p
    return out.reshape(B, T, NN, C).astype(np.float32)


# revision 18
# speedup vs baseline: 1.3681x; 1.0229x over previous
"""Trainium2 Bass kernel for nn_Diff_prop_18425409699925 (GNN message passing).

Math (per batch element b, with x = local_feat[b] reshaped to [n=1024, c=256]):
  xn   = x / ||x||_row
  A    = (xn * diag(W_adj)) @ xn^T                (symmetric; einsum uses only
                                                   the diagonal of W_adj)
  G    = exp(5*A) with diagonal zeroed            (the reference's row-max
                                                   shift cancels exactly in the
                                                   row-normalized mean)
  M    = (G @ x) / rowsum(G)
  diff = (x - M) @ W_aff^T + b_aff
  y    = LeakyReLU(LayerNorm(diff) * gamma + beta, 0.01)

Sharding: data-parallel over batch B=8 -> one batch element per NeuronCore,
weights replicated, no collectives. G (symmetric) is used directly as the
lhsT of the G @ x matmul, avoiding a [1024,1024] transpose.

All matmuls and big elementwise ops run in bf16 (operands; PSUM accumulation
is fp32).  Every ScalarE activation in the kernel (Exp, Ln, Square, Copy,
Prelu) lives in the single `natural_log_exp_and_others` table set, so the
~2.7us ACT table load is paid exactly once; sqrt/rsqrt/reciprocal are
computed as exp(-k*ln(s)).

The row-sum of diff (needed for the LayerNorm mean) is obtained for free by
appending the column-sums of W as an extra matmul column.  The sign of W is
flipped host-side so the on-device D' = M - x (natural STT operand order)
yields diff = (x - M) @ W^T.

global_feat and pos are unused by the reference; accepted and ignored.
"""

import os
import sys

import numpy as np

for _p in ("/opt/trn_rl_repo",):
    if os.path.isdir(_p) and _p not in sys.path:
        sys.path.insert(0, _p)

import ml_dtypes
import concourse.bacc as bacc
import concourse.bass as bass
import concourse.tile as tile
from concourse import mybir
from concourse.bass_utils import run_bass_kernel_spmd

B, T, NN, C = 8, 16, 64, 256
N = T * NN            # 1024 nodes per batch element
P = 128               # partitions
NT = N // P           # 8 n-tiles
CT = C // P           # 2 c-tiles
CX = C + 1            # D@W output incl. the rowsum column
F32 = mybir.dt.float32
BF16 = mybir.dt.bfloat16
TS = bass.ts
BF = ml_dtypes.bfloat16

LN_EPS = 1e-5
LEAK = 0.01
DIAG_NEG = -200.0     # added to diagonal of A pre-exp -> exp underflows to 0


def _steered_act_tables(orig_fn):
    """Steer the ACT table-set chooser: Exp and Ln both live in
    `natural_log_exp_and_others`, but the greedy chooser maps each function
    to the FIRST set containing it (natural_log for Ln, exp_and_others for
    Exp), thrashing a ~2.7us table load on every Ln<->Exp alternation.
    Emptying every other set makes the chooser settle on the one set that
    contains all activations this kernel uses -> exactly one load.  Set
    indices (and hence the emitted act_func_set_id) are unchanged."""
    def wrapped(arch):
        t = orig_fn(arch)
        AF = mybir.ActivationFunctionType
        if "natural_log_exp_and_others" in t:
            for nm in t:
                if nm != "natural_log_exp_and_others":
                    t[nm] = set()
        return t
    return wrapped


def _build_program(diag_one, ln_trivial):
    nc = bacc.Bacc("TRN2", target_bir_lowering=False, debug=False)
    _orig_gat = bacc.get_activation_tables
    bacc.get_activation_tables = _steered_act_tables(_orig_gat)
    try:
        return _build_program_inner(nc, diag_one, ln_trivial)
    finally:
        bacc.get_activation_tables = _orig_gat


def _build_program_inner(nc, diag_one, ln_trivial):

    xb_d = nc.declare_dram_parameter("xb", [P, NT, CX], BF16, isOutput=False)
    wt_d = nc.declare_dram_parameter("wt", [P, CT, CX], BF16, isOutput=False)
    identb_d = nc.declare_dram_parameter("identb", [P, P], BF16, isOutput=False)
    wdiag_d = nc.declare_dram_parameter("wdiag", [P, CT], F32, isOutput=False)
    rows_d = nc.declare_dram_parameter("rows", [1, 2 * C], F32, isOutput=False)
    bvec_d = nc.declare_dram_parameter("bvec", [1, CX], BF16, isOutput=False)
    y_d = nc.declare_dram_parameter("y", [P, NT, C], F32, isOutput=True)

    with tile.TileContext(nc) as tc:
        _emit(nc, tc, xb_d, wt_d, identb_d, wdiag_d, rows_d, bvec_d, y_d,
              diag_one, ln_trivial)
    nc.finalize()
    return nc


def _emit(nc, tc, xb_d, wt_d, identb_d, wdiag_d, rows_d, bvec_d, y_d,
          diag_one, ln_trivial):
    from contextlib import ExitStack

    mult = mybir.AluOpType.mult
    add = mybir.AluOpType.add
    subtract = mybir.AluOpType.subtract
    bypass = mybir.AluOpType.bypass
    amax = mybir.AluOpType.max
    lshr = mybir.AluOpType.logical_shift_right
    bnot = mybir.AluOpType.bitwise_not
    AF = mybir.ActivationFunctionType
    U32 = mybir.dt.uint32

    W0 = int(os.environ.get("KERNEL_WARMUP", "28"))
    WA = int(os.environ.get("KERNEL_WARMUP_A", "1"))
    WB = int(os.environ.get("KERNEL_WARMUP_B", "2"))
    WC = int(os.environ.get("KERNEL_WARMUP_C", "12"))
    use_prelu = bool(int(os.environ.get("KERNEL_PRELU", "1")))

    v = nc.vector
    s = nc.scalar
    te = nc.tensor
    sy = nc.sync
    gp = nc.gpsimd
    dma_eng = [sy, nc.scalar, gp]

    with ExitStack() as ctx:
        sb = ctx.enter_context(tc.tile_pool(name="sb", bufs=1))
        scr = ctx.enter_context(tc.tile_pool(name="scr", bufs=3))
        ps_a = ctx.enter_context(tc.tile_pool(name="ps_a", bufs=2, space="PSUM"))
        ps_c = ctx.enter_context(tc.tile_pool(name="ps_c", bufs=3, space="PSUM"))
        ps_t = ctx.enter_context(tc.tile_pool(name="ps_t", bufs=1, space="PSUM"))

        # ---------------- persistent SBUF tiles ----------------
        # Xb carries a trailing ones column so the G @ x matmul also yields
        # rowsum(G) (the L1 normalizer) in column 256 -- no reduction pass.
        Xb = sb.tile([P, NT, CX], BF16, tag="Xb", name="Xb")
        xnT = sb.tile([P, CT, N], BF16, tag="xnT", name="xnT")
        if diag_one:
            xnTs = xnT
        else:
            xnTs = sb.tile([P, CT, N], BF16, tag="xnTs", name="xnTs")
        G = [sb.tile([P, N], BF16, tag=f"G{i}", name=f"G{i}") for i in range(NT)]
        DTt = sb.tile([P, CT, N], BF16, tag="DTt", name="DTt")
        Y = sb.tile([P, NT, C], F32, tag="Y", name="Y")
        WT = sb.tile([P, CT, CX], BF16, tag="WT", name="WT")
        identb = sb.tile([P, P], BF16, tag="identb", name="identb")
        negeyeb = sb.tile([P, P], BF16, tag="negeyeb", name="negeyeb")
        warm_src = sb.tile([P, C], BF16, tag="warm_src", name="warm_src")

        # batched per-row stats, one column per n-tile
        def stat(nm):
            return sb.tile([P, NT], F32, tag=nm, name=nm)
        SS, LNS, RNO = stat("SS"), stat("LNS"), stat("RNO")
        NRS = stat("NRS")
        SQ, MUn, MUSQ = stat("SQ"), stat("MUn"), stat("MUSQ")
        SQA, VAR, LV, RSTD, NB = (stat("SQA"), stat("VAR"), stat("LV"),
                                  stat("RSTD"), stat("NB"))

        if not diag_one:
            wdiag = sb.tile([P, CT], F32, tag="wdiag", name="wdiag")
        if not ln_trivial:
            g_bc = sb.tile([P, C], F32, tag="g_bc", name="g_bc")
            be_bc = sb.tile([P, C], F32, tag="be_bc", name="be_bc")
            rows = sb.tile([1, 2 * C], F32, tag="rows", name="rows")
            bvec = sb.tile([1, CX], BF16, tag="bvec", name="bvec")
            ones1f = sb.tile([1, P], F32, tag="ones1f", name="ones1f")
            ones1b = sb.tile([1, P], BF16, tag="ones1b", name="ones1b")

        # ---------------- loads (spread across DGE queues) ----------------
        # xb/wt/y are partition-major in DRAM (host handles the reshuffle),
        # so every partition's data is one contiguous descriptor.
        sy.dma_start(identb[:], identb_d[:])
        gp.dma_start(WT[:], wt_d[:])
        if not diag_one:
            gp.dma_start(wdiag[:], wdiag_d[:])
        if not ln_trivial:
            gp.dma_start(rows[:], rows_d[:])
            gp.dma_start(bvec[:], bvec_d[:])
        h = NT // 2
        sy.dma_start(Xb[:, 0:h, :], xb_d[:, 0:h, :])
        nc.scalar.dma_start(Xb[:, h:NT, :], xb_d[:, h:NT, :])

        v.memset(warm_src[:], 0.0)
        v.tensor_scalar_mul(negeyeb[:], identb[:], DIAG_NEG)

        if not ln_trivial:
            v.memset(ones1f[:], 1.0)
            v.memset(ones1b[:], 1.0)
            pg = ps_a.tile([P, N], F32, tag="pa", name="pg")
            nc.tensor.matmul(pg[:, 0:2 * C], ones1f[:], rows[:],
                             start=True, stop=True)
            v.tensor_copy(g_bc[:], pg[:, 0:C])
            v.tensor_copy(be_bc[:], pg[:, C:2 * C])

        # PE warm-up: keep the HAM clock gate open while the PE would
        # otherwise idle (input-DMA ramp, exp drain), so matmuls run at
        # 2.4 GHz instead of the cold 1.2 GHz.
        pw = None

        def warm(n):
            nonlocal pw
            if pw is None:
                pw = ps_a.tile([P, N], F32, tag="pa", name="pw")
            for _ in range(n):
                te.matmul(pw[:, 0:C], warm_src[:, 0:P], warm_src[:],
                          start=True, stop=True)

        warm(W0)

        ptbank = ps_t.tile([P, 2, CT, P], BF16, tag="pt", name="ptbank")
        pts = [ptbank[:, 0, :, :], ptbank[:, 1, :, :]]

        # ---------------- phase A: row-normalize, build xn^T ----------------
        # two half-batches so tiles 0-3 process while tiles 4-7 still load;
        # rsqrt = exp(-0.5*ln) on ScalarE, batched per half
        for hh in range(2):
            lo, hi = hh * (NT // 2), (hh + 1) * (NT // 2)
            for i in range(lo, hi):
                sqs = scr.tile([P, C], BF16, tag="sqs", name="sqs")
                v.scalar_tensor_tensor(
                    out=sqs[:], in0=Xb[:, i, 0:C], scalar=1.0,
                    in1=Xb[:, i, 0:C],
                    op0=bypass, op1=mult, accum_out=SS[:, i:i + 1])
            s.activation(LNS[:, lo:hi], SS[:, lo:hi], AF.Ln)
            s.activation(RNO[:, lo:hi], LNS[:, lo:hi], AF.Exp, scale=-0.5)
            xns = {}
            for i in range(lo, hi):
                xn = scr.tile([P, C], BF16, tag="xn", name=f"xn{i}",
                              bufs=4)
                v.tensor_scalar_mul(xn[:], Xb[:, i, 0:C], RNO[:, i:i + 1])
                xns[i] = xn
            for i in range(lo, hi):
                # two slots inside one PSUM bank double-buffer the
                # transpose -> copy ping-pong
                sl = i % 2
                pt = pts[sl]
                for k in range(CT):
                    te.transpose(pt[:, k, :], xns[i][:, TS(k, P)], identb[:])
                warm(WA)
                dst = xnT[:, :, TS(i, P)]
                if i % 2 == 0:
                    s.activation(dst, pt[:], AF.Copy)
                else:
                    v.tensor_copy(dst, pt[:])
                if not diag_one:
                    for k in range(CT):
                        v.tensor_scalar_mul(
                            xnTs[:, k, TS(i, P)], pt[:, k, :],
                            wdiag[:, k:k + 1])

        # ---------------- phase B: A = xnTs^T @ xnT, G = exp(5A) ----------------
        # The -200*I diagonal knockout is an extra accumulating matmul, so
        # exp depends only on the PE and ScalarE streams exp back-to-back.
        for i in range(NT):
            jd = i // 4
            pa = ps_a.tile([P, N], F32, tag="pa", name=f"pa{i}")
            for j in range(2):
                for k in range(CT):
                    te.matmul(
                        pa[:, TS(j, 512)],
                        xnTs[:, k, TS(i, P)],
                        xnT[:, k, TS(j, 512)],
                        start=(k == 0),
                        stop=(k == CT - 1) and (j != jd))
            te.matmul(pa[:, TS(i, P)], negeyeb[:], identb[:],
                      start=False, stop=True)
            warm(WB)
            s.activation(G[i][:], pa[:], AF.Exp, scale=5.0)
        warm(WC)

        # ---------------- phase C: M, diff = (x-M)W^T, LN, LeakyReLU ----------------
        # 3-stage software pipeline with a 2-block skew: S1 = G@x + 1/rowsum
        # + D', S2 (two blocks later) = transpose + D@W + Square + Prelu(pd-mu),
        # S3 (per 4-tile group) = batched rstd + scale + store.  LeakyReLU is
        # positively homogeneous, so the 1/std scaling happens after the
        # activation on DVE and pd's PSUM lifetime ends inside S2.  Engine
        # queues are ordered by data readiness (transpose first on PE, copy
        # first on DVE) so nothing head-blocks.
        y_g = y_d
        Dbs = [None] * NT
        qs = [None] * NT
        qpool = ctx.enter_context(tc.tile_pool(name="qpool", bufs=5))

        def finish_group(i0, n):
            s.activation(LV[:, i0:i0 + n], VAR[:, i0:i0 + n], AF.Ln)
            s.activation(RSTD[:, i0:i0 + n], LV[:, i0:i0 + n], AF.Exp,
                         scale=-0.5)
            for i in range(i0, i0 + n):
                if ln_trivial:
                    v.tensor_scalar_mul(Y[:, i, :], qs[i], RSTD[:, i:i + 1])
                else:
                    # q = diff - mu; t = q*rstd, then gamma/beta + leaky
                    u = scr.tile([P, C], F32, tag="u", name="u")
                    v.tensor_scalar_mul(u[:], qs[i], RSTD[:, i:i + 1])
                    uu = scr.tile([P, C], F32, tag="uu", name="uu")
                    v.tensor_mul(uu[:], u[:], g_bc[:])
                    w_ = scr.tile([P, C], F32, tag="w_", name="w_")
                    v.tensor_add(w_[:], uu[:], be_bc[:])
                    v.scalar_tensor_tensor(
                        out=Y[:, i, :], in0=w_[:], scalar=LEAK, in1=w_[:],
                        op0=mult, op1=amax)
                sy.dma_start(y_g[:, i, :], Y[:, i, :])

        for ii in range(NT + 2):
            j = ii - 2  # S2 tile index
            if 0 <= j:
                # PE: transpose first -- Db_j has been ready for a block
                Db = Dbs[j]
                ptd = pts[j % 2]
                for k in range(CT):
                    te.transpose(ptd[:, k, :], Db[:, TS(k, P)], identb[:])
            if ii < NT:
                i = ii
                py = ps_c.tile([P, CX], F32, tag="pc", name=f"py{i}")
                for k in range(NT):
                    te.matmul(py[:], G[k][:, TS(i, P)], Xb[:, k, :],
                              start=(k == 0), stop=(k == NT - 1))
            if 0 <= j:
                v.tensor_copy(DTt[:, :, TS(j, P)], ptd[:])
                pd = ps_c.tile([P, CX], F32, tag="pc", name=f"pd{j}")
                have_b = not ln_trivial
                for k in range(CT):
                    te.matmul(pd[:], DTt[:, k, TS(j, P)], WT[:, k, :],
                              start=(k == 0),
                              stop=(k == CT - 1) and not have_b)
                if have_b:
                    te.matmul(pd[:], ones1b[:], bvec[:], start=False,
                              stop=True)
                sqo = scr.tile([P, C], F32, tag="sqo", name="sqo")
                s.activation(sqo[:], pd[:, 0:C], AF.Square,
                             accum_out=SQ[:, j:j + 1])
            if ii < NT:
                i = ii
                # py[:,256] = rowsum(G_i); NRS = 1/rowsum
                v.reciprocal(NRS[:, i:i + 1], py[:, C:CX])
            if 0 <= j:
                v.tensor_scalar_mul(MUn[:, j:j + 1], pd[:, C:CX], -1.0 / C)
                v.tensor_mul(MUSQ[:, j:j + 1], MUn[:, j:j + 1],
                             MUn[:, j:j + 1])
                v.tensor_scalar(
                    out=SQA[:, j:j + 1], in0=SQ[:, j:j + 1],
                    scalar1=1.0 / C, scalar2=LN_EPS, op0=mult, op1=add)
                v.scalar_tensor_tensor(
                    out=VAR[:, j:j + 1], in0=SQA[:, j:j + 1], scalar=1.0,
                    in1=MUSQ[:, j:j + 1], op0=bypass, op1=subtract)
                q = qpool.tile([P, C], F32, tag="q", name=f"q{j}")
                qs[j] = q
                if ln_trivial and use_prelu:
                    s.activation(q[:], pd[:, 0:C], AF.Prelu,
                                 bias=MUn[:, j:j + 1], alpha=LEAK)
                elif ln_trivial:
                    # CoreSim fallback: Prelu isn't implemented there
                    tt = scr.tile([P, C], F32, tag="tt", name="tt")
                    s.activation(tt[:], pd[:, 0:C], AF.Identity,
                                 bias=MUn[:, j:j + 1])
                    v.scalar_tensor_tensor(
                        out=q[:], in0=tt[:], scalar=LEAK, in1=tt[:],
                        op0=mult, op1=amax)
                else:
                    # generic path: keep (diff - mu) unactivated; gamma/beta
                    # and the leaky relu are applied in finish_group
                    s.activation(q[:], pd[:, 0:C], AF.Identity,
                                 bias=MUn[:, j:j + 1])
            if ii < NT:
                i = ii
                # D' = M - x  (sign of W is pre-flipped so diff = (x-M)W^T)
                Db = scr.tile([P, C], BF16, tag="db", name=f"db{i}")
                v.scalar_tensor_tensor(
                    out=Db[:], in0=py[:, 0:C], scalar=NRS[:, i:i + 1],
                    in1=Xb[:, i, 0:C], op0=mult, op1=subtract)
                Dbs[i] = Db
            if j == 3:
                finish_group(0, 4)
            elif j in (5, 7):
                finish_group(j - 1, 2)


_PROGRAM_CACHE = {}
last_results = None


def _get_program(diag_one=True, ln_trivial=True):
    key = (diag_one, ln_trivial,
           os.environ.get("KERNEL_WARMUP", "48"),
           os.environ.get("KERNEL_WARMUP_A", "2"),
           os.environ.get("KERNEL_WARMUP_B", "2"),
           os.environ.get("KERNEL_WARMUP_C", "12"),
           os.environ.get("KERNEL_PRELU", "1"))
    if key not in _PROGRAM_CACHE:
        _PROGRAM_CACHE[key] = _build_program(diag_one, ln_trivial)
    return _PROGRAM_CACHE[key]


def _prep_inputs(local_feat, W_adj, W_aff, b_aff, ln_gamma, ln_beta):
    x = np.asarray(local_feat, np.float32).reshape(B, N, C)
    # partition-major layout with a trailing ones column: xb_r[b, p, i, :] =
    # [x[b, i*128+p, :], 1.0] -- every SBUF partition's data is one
    # contiguous DMA descriptor, and the ones column makes the G @ x matmul
    # also produce rowsum(G).
    xb = np.ones((B, P, NT, CX), dtype=BF)
    xb[:, :, :, 0:C] = x.reshape(B, NT, P, C).transpose(0, 2, 1, 3).astype(BF)
    Wf = np.asarray(W_aff, np.float32)
    wneg = np.ascontiguousarray(-Wf.T).astype(BF)        # [cin, cout]
    w1 = wneg.astype(np.float32).sum(axis=1).astype(BF)  # rowsum column
    wt = np.concatenate([wneg, w1[:, None]], axis=1)     # [C, 257] bf16
    wt_r = np.ascontiguousarray(
        wt.reshape(CT, P, CX).transpose(1, 0, 2))        # [P, CT, 257]
    identb = np.eye(P, dtype=np.float32).astype(BF)
    diag = np.ascontiguousarray(np.diagonal(np.asarray(W_adj, np.float32)))
    wd = np.ascontiguousarray(diag.reshape(CT, P).T).astype(np.float32)
    b = np.asarray(b_aff, np.float32).ravel()
    g = np.asarray(ln_gamma, np.float32).ravel()
    be = np.asarray(ln_beta, np.float32).ravel()
    rows = np.concatenate([g, be]).reshape(1, 2 * C).astype(np.float32)
    bvec = np.concatenate([b, [b.sum()]]).reshape(1, CX).astype(BF)
    diag_one = bool(np.all(diag == 1.0))
    ln_trivial = bool(np.all(g == 1.0) and np.all(be == 0.0)
                      and np.all(b == 0.0))
    in_maps = [
        {"xb": np.ascontiguousarray(xb[bb]), "wt": wt_r, "identb": identb,
         "wdiag": wd, "rows": rows, "bvec": bvec}
        for bb in range(B)
    ]
    return in_maps, diag_one, ln_trivial


def kernel(local_feat, global_feat, pos, W_adj, W_aff, b_aff, ln_gamma,
           ln_beta, **_unused):
    global last_results
    in_maps, diag_one, ln_trivial = _prep_inputs(
        local_feat, W_adj, W_aff, b_aff, ln_gamma, ln_beta)
    nc = _get_program(diag_one, ln_trivial)
    trace = bool(int(os.environ.get("KERNEL_TRACE", "0")))
    res = run_bass_kernel_spmd(nc, in_maps, list(range(B)), trace=trace)
    last_results = res
    out = np.stack([np.asarray(res.results[bb]["y"]) for bb in range(B)],
                   axis=0)                                # [B, P, NT, C]
    out = out.transpose(0, 2, 1, 3).reshape(B, N, C)      # n = i*128 + p
    return out.reshape(B, T, NN, C).astype(np.float32)


# revision 19
# speedup vs baseline: 1.4189x; 1.0371x over previous
"""Trainium2 Bass kernel for nn_Diff_prop_18425409699925 (GNN message passing).

Math (per batch element b, with x = local_feat[b] reshaped to [n=1024, c=256]):
  xn   = x / ||x||_row
  A    = (xn * diag(W_adj)) @ xn^T                (symmetric; einsum uses only
                                                   the diagonal of W_adj)
  G    = exp(5*A) with diagonal zeroed            (the reference's row-max
                                                   shift cancels exactly in the
                                                   row-normalized mean)
  M    = (G @ x) / rowsum(G)
  diff = (x - M) @ W_aff^T + b_aff
  y    = LeakyReLU(LayerNorm(diff) * gamma + beta, 0.01)

Sharding: data-parallel over batch B=8 -> one batch element per NeuronCore,
weights replicated, no collectives. G (symmetric) is used directly as the
lhsT of the G @ x matmul, avoiding a [1024,1024] transpose.

All matmuls and big elementwise ops run in bf16 (operands; PSUM accumulation
is fp32).  Every ScalarE activation in the kernel (Exp, Ln, Square, Copy,
Prelu) lives in the single `natural_log_exp_and_others` table set, so the
~2.7us ACT table load is paid exactly once; sqrt/rsqrt/reciprocal are
computed as exp(-k*ln(s)).

The row-sum of diff (needed for the LayerNorm mean) is obtained for free by
appending the column-sums of W as an extra matmul column.  The sign of W is
flipped host-side so the on-device D' = M - x (natural STT operand order)
yields diff = (x - M) @ W^T.

global_feat and pos are unused by the reference; accepted and ignored.
"""

import os
import sys

import numpy as np

for _p in ("/opt/trn_rl_repo",):
    if os.path.isdir(_p) and _p not in sys.path:
        sys.path.insert(0, _p)

import ml_dtypes
import concourse.bacc as bacc
import concourse.bass as bass
import concourse.tile as tile
from concourse import mybir
from concourse.bass_utils import run_bass_kernel_spmd

B, T, NN, C = 8, 16, 64, 256
N = T * NN            # 1024 nodes per batch element
P = 128               # partitions
NT = N // P           # 8 n-tiles
CT = C // P           # 2 c-tiles
CX = C + 1            # D@W output incl. the rowsum column
F32 = mybir.dt.float32
BF16 = mybir.dt.bfloat16
TS = bass.ts
BF = ml_dtypes.bfloat16

LN_EPS = 1e-5
LEAK = 0.01
DIAG_NEG = -200.0     # added to diagonal of A pre-exp -> exp underflows to 0


def _steered_act_tables(orig_fn):
    """Steer the ACT table-set chooser: Exp and Ln both live in
    `natural_log_exp_and_others`, but the greedy chooser maps each function
    to the FIRST set containing it (natural_log for Ln, exp_and_others for
    Exp), thrashing a ~2.7us table load on every Ln<->Exp alternation.
    Emptying every other set makes the chooser settle on the one set that
    contains all activations this kernel uses -> exactly one load.  Set
    indices (and hence the emitted act_func_set_id) are unchanged."""
    def wrapped(arch):
        t = orig_fn(arch)
        AF = mybir.ActivationFunctionType
        if "natural_log_exp_and_others" in t:
            for nm in t:
                if nm != "natural_log_exp_and_others":
                    t[nm] = set()
        return t
    return wrapped


def _build_program(diag_one, ln_trivial):
    nc = bacc.Bacc("TRN2", target_bir_lowering=False, debug=False)
    _orig_gat = bacc.get_activation_tables
    bacc.get_activation_tables = _steered_act_tables(_orig_gat)
    try:
        return _build_program_inner(nc, diag_one, ln_trivial)
    finally:
        bacc.get_activation_tables = _orig_gat


def _build_program_inner(nc, diag_one, ln_trivial):

    xb_d = nc.declare_dram_parameter("xb", [P, NT, CX], BF16, isOutput=False)
    wt_d = nc.declare_dram_parameter("wt", [P, CT, CX], BF16, isOutput=False)
    identb_d = nc.declare_dram_parameter("identb", [P, P], BF16, isOutput=False)
    wdiag_d = nc.declare_dram_parameter("wdiag", [P, CT], F32, isOutput=False)
    rows_d = nc.declare_dram_parameter("rows", [1, 2 * C], F32, isOutput=False)
    bvec_d = nc.declare_dram_parameter("bvec", [1, CX], BF16, isOutput=False)
    y_d = nc.declare_dram_parameter("y", [P, NT, C], F32, isOutput=True)

    with tile.TileContext(nc) as tc:
        _emit(nc, tc, xb_d, wt_d, identb_d, wdiag_d, rows_d, bvec_d, y_d,
              diag_one, ln_trivial)
    nc.finalize()
    return nc


def _emit(nc, tc, xb_d, wt_d, identb_d, wdiag_d, rows_d, bvec_d, y_d,
          diag_one, ln_trivial):
    from contextlib import ExitStack

    mult = mybir.AluOpType.mult
    add = mybir.AluOpType.add
    subtract = mybir.AluOpType.subtract
    bypass = mybir.AluOpType.bypass
    amax = mybir.AluOpType.max
    lshr = mybir.AluOpType.logical_shift_right
    bnot = mybir.AluOpType.bitwise_not
    AF = mybir.ActivationFunctionType
    U32 = mybir.dt.uint32

    W0 = int(os.environ.get("KERNEL_WARMUP", "28"))
    WA = int(os.environ.get("KERNEL_WARMUP_A", "0"))
    WB = int(os.environ.get("KERNEL_WARMUP_B", "2"))
    WC = int(os.environ.get("KERNEL_WARMUP_C", "12"))
    use_prelu = bool(int(os.environ.get("KERNEL_PRELU", "1")))

    v = nc.vector
    s = nc.scalar
    te = nc.tensor
    sy = nc.sync
    gp = nc.gpsimd
    dma_eng = [sy, nc.scalar, gp]

    with ExitStack() as ctx:
        sb = ctx.enter_context(tc.tile_pool(name="sb", bufs=1))
        scr = ctx.enter_context(tc.tile_pool(name="scr", bufs=3))
        ps_a = ctx.enter_context(tc.tile_pool(name="ps_a", bufs=2, space="PSUM"))
        ps_c = ctx.enter_context(tc.tile_pool(name="ps_c", bufs=2, space="PSUM"))
        ps_t = ctx.enter_context(tc.tile_pool(name="ps_t", bufs=2, space="PSUM"))

        # ---------------- persistent SBUF tiles ----------------
        # Xb carries a trailing ones column so the G @ x matmul also yields
        # rowsum(G) (the L1 normalizer) in column 256 -- no reduction pass.
        Xb = sb.tile([P, NT, CX], BF16, tag="Xb", name="Xb")
        xnT = sb.tile([P, CT, N], BF16, tag="xnT", name="xnT")
        if diag_one:
            xnTs = xnT
        else:
            xnTs = sb.tile([P, CT, N], BF16, tag="xnTs", name="xnTs")
        G = [sb.tile([P, N], BF16, tag=f"G{i}", name=f"G{i}") for i in range(NT)]
        DTt = sb.tile([P, CT, N], BF16, tag="DTt", name="DTt")
        Y = sb.tile([P, NT, C], F32, tag="Y", name="Y")
        WT = sb.tile([P, CT, CX], BF16, tag="WT", name="WT")
        identb = sb.tile([P, P], BF16, tag="identb", name="identb")
        negeyeb = sb.tile([P, P], BF16, tag="negeyeb", name="negeyeb")
        warm_src = sb.tile([P, C], BF16, tag="warm_src", name="warm_src")

        # batched per-row stats, one column per n-tile
        def stat(nm):
            return sb.tile([P, NT], F32, tag=nm, name=nm)
        SS, LNS, RNO = stat("SS"), stat("LNS"), stat("RNO")
        NRS = stat("NRS")
        SQ, MUn, MUSQ = stat("SQ"), stat("MUn"), stat("MUSQ")
        SQA, VAR, LV, RSTD, NB = (stat("SQA"), stat("VAR"), stat("LV"),
                                  stat("RSTD"), stat("NB"))

        if not diag_one:
            wdiag = sb.tile([P, CT], F32, tag="wdiag", name="wdiag")
        if not ln_trivial:
            g_bc = sb.tile([P, C], F32, tag="g_bc", name="g_bc")
            be_bc = sb.tile([P, C], F32, tag="be_bc", name="be_bc")
            rows = sb.tile([1, 2 * C], F32, tag="rows", name="rows")
            bvec = sb.tile([1, CX], BF16, tag="bvec", name="bvec")
            ones1f = sb.tile([1, P], F32, tag="ones1f", name="ones1f")
            ones1b = sb.tile([1, P], BF16, tag="ones1b", name="ones1b")

        # ---------------- loads (spread across DGE queues) ----------------
        # xb/wt/y are partition-major in DRAM (host handles the reshuffle),
        # so every partition's data is one contiguous descriptor.
        sy.dma_start(identb[:], identb_d[:])
        gp.dma_start(WT[:], wt_d[:])
        if not diag_one:
            gp.dma_start(wdiag[:], wdiag_d[:])
        if not ln_trivial:
            gp.dma_start(rows[:], rows_d[:])
            gp.dma_start(bvec[:], bvec_d[:])
        for qq in range(4):
            lo, hi = qq * 2, qq * 2 + 2
            (sy if qq % 2 == 0 else nc.scalar).dma_start(
                Xb[:, lo:hi, :], xb_d[:, lo:hi, :])

        v.memset(warm_src[:], 0.0)
        v.tensor_scalar_mul(negeyeb[:], identb[:], DIAG_NEG)

        if not ln_trivial:
            v.memset(ones1f[:], 1.0)
            v.memset(ones1b[:], 1.0)
            pg = ps_a.tile([P, N], F32, tag="pa", name="pg")
            nc.tensor.matmul(pg[:, 0:2 * C], ones1f[:], rows[:],
                             start=True, stop=True)
            v.tensor_copy(g_bc[:], pg[:, 0:C])
            v.tensor_copy(be_bc[:], pg[:, C:2 * C])

        # PE warm-up: keep the HAM clock gate open while the PE would
        # otherwise idle (input-DMA ramp, exp drain), so matmuls run at
        # 2.4 GHz instead of the cold 1.2 GHz.
        pw = None

        def warm(n):
            nonlocal pw
            if pw is None:
                pw = ps_a.tile([P, N], F32, tag="pa", name="pw")
            for _ in range(n):
                te.matmul(pw[:, 0:C], warm_src[:, 0:P], warm_src[:],
                          start=True, stop=True)

        warm(W0)

        # ---------------- phase A: row-normalize, build xn^T ----------------
        # quarter-batches track the 4 input-DMA chunks; rsqrt =
        # exp(-0.5*ln) on ScalarE, batched per quarter
        for hh in range(4):
            lo, hi = hh * 2, hh * 2 + 2
            for i in range(lo, hi):
                sqs = scr.tile([P, C], BF16, tag="sqs", name="sqs")
                v.scalar_tensor_tensor(
                    out=sqs[:], in0=Xb[:, i, 0:C], scalar=1.0,
                    in1=Xb[:, i, 0:C],
                    op0=bypass, op1=mult, accum_out=SS[:, i:i + 1])
            s.activation(LNS[:, lo:hi], SS[:, lo:hi], AF.Ln)
            s.activation(RNO[:, lo:hi], LNS[:, lo:hi], AF.Exp, scale=-0.5)
            for i in range(lo, hi):
                xn = scr.tile([P, C], BF16, tag="xn", name=f"xn{i}",
                              bufs=3)
                v.tensor_scalar_mul(xn[:], Xb[:, i, 0:C], RNO[:, i:i + 1])
                pt = ps_t.tile([P, CT, P], BF16, tag="pt", name="pt")
                for k in range(CT):
                    te.transpose(pt[:, k, :], xn[:, TS(k, P)], identb[:])
                warm(WA)
                dst = xnT[:, :, TS(i, P)]
                if i % 2 == 0:
                    s.activation(dst, pt[:], AF.Copy)
                else:
                    v.tensor_copy(dst, pt[:])
                if not diag_one:
                    for k in range(CT):
                        v.tensor_scalar_mul(
                            xnTs[:, k, TS(i, P)], pt[:, k, :],
                            wdiag[:, k:k + 1])

        # ---------------- phase B: A = xnTs^T @ xnT, G = exp(5A) ----------------
        # The -200*I diagonal knockout is an extra accumulating matmul, so
        # exp depends only on the PE and ScalarE streams exp back-to-back.
        for i in range(NT):
            jd = i // 4
            pa = ps_a.tile([P, N], F32, tag="pa", name=f"pa{i}")
            for j in range(2):
                for k in range(CT):
                    te.matmul(
                        pa[:, TS(j, 512)],
                        xnTs[:, k, TS(i, P)],
                        xnT[:, k, TS(j, 512)],
                        start=(k == 0),
                        stop=(k == CT - 1) and (j != jd))
            te.matmul(pa[:, TS(i, P)], negeyeb[:], identb[:],
                      start=False, stop=True)
            warm(WB)
            s.activation(G[i][:], pa[:], AF.Exp, scale=5.0)
        warm(WC)

        # ---------------- phase C: M, diff = (x-M)W^T, LN, LeakyReLU ----------------
        # 3-stage software pipeline with a 2-block skew: S1 = G@x + 1/rowsum
        # + D', S2 (two blocks later) = transpose + D@W + Square + Prelu(pd-mu),
        # S3 (per 4-tile group) = batched rstd + scale + store.  LeakyReLU is
        # positively homogeneous, so the 1/std scaling happens after the
        # activation on DVE and pd's PSUM lifetime ends inside S2.  Engine
        # queues are ordered by data readiness (transpose first on PE, copy
        # first on DVE) so nothing head-blocks.
        y_g = y_d
        Dbs = [None] * NT
        qs = [None] * NT
        qpool = ctx.enter_context(tc.tile_pool(name="qpool", bufs=5))

        def finish_group(i0, n):
            s.activation(LV[:, i0:i0 + n], VAR[:, i0:i0 + n], AF.Ln)
            s.activation(RSTD[:, i0:i0 + n], LV[:, i0:i0 + n], AF.Exp,
                         scale=-0.5)
            for i in range(i0, i0 + n):
                if ln_trivial:
                    v.tensor_scalar_mul(Y[:, i, :], qs[i], RSTD[:, i:i + 1])
                else:
                    # q = diff - mu; t = q*rstd, then gamma/beta + leaky
                    u = scr.tile([P, C], F32, tag="u", name="u")
                    v.tensor_scalar_mul(u[:], qs[i], RSTD[:, i:i + 1])
                    uu = scr.tile([P, C], F32, tag="uu", name="uu")
                    v.tensor_mul(uu[:], u[:], g_bc[:])
                    w_ = scr.tile([P, C], F32, tag="w_", name="w_")
                    v.tensor_add(w_[:], uu[:], be_bc[:])
                    v.scalar_tensor_tensor(
                        out=Y[:, i, :], in0=w_[:], scalar=LEAK, in1=w_[:],
                        op0=mult, op1=amax)
                sy.dma_start(y_g[:, i, :], Y[:, i, :])

        for ii in range(NT + 2):
            j = ii - 2  # S2 tile index
            if 0 <= j:
                # PE: transpose first -- Db_j has been ready for a block
                Db = Dbs[j]
                ptd = ps_t.tile([P, CT, P], BF16, tag="pt", name=f"ptd{j}")
                for k in range(CT):
                    te.transpose(ptd[:, k, :], Db[:, TS(k, P)], identb[:])
            if ii < NT:
                i = ii
                py = ps_c.tile([P, CX], F32, tag="pc", name=f"py{i}")
                for k in range(NT):
                    te.matmul(py[:], G[k][:, TS(i, P)], Xb[:, k, :],
                              start=(k == 0), stop=(k == NT - 1))
            if 0 <= j:
                v.tensor_copy(DTt[:, :, TS(j, P)], ptd[:])
                pd = ps_c.tile([P, CX], F32, tag="pc", name=f"pd{j}")
                have_b = not ln_trivial
                for k in range(CT):
                    te.matmul(pd[:], DTt[:, k, TS(j, P)], WT[:, k, :],
                              start=(k == 0),
                              stop=(k == CT - 1) and not have_b)
                if have_b:
                    te.matmul(pd[:], ones1b[:], bvec[:], start=False,
                              stop=True)
                sqo = scr.tile([P, C], F32, tag="sqo", name="sqo")
                s.activation(sqo[:], pd[:, 0:C], AF.Square,
                             accum_out=SQ[:, j:j + 1])
            if ii < NT:
                i = ii
                # py[:,256] = rowsum(G_i); NRS = 1/rowsum
                v.reciprocal(NRS[:, i:i + 1], py[:, C:CX])
            if 0 <= j:
                v.tensor_scalar_mul(MUn[:, j:j + 1], pd[:, C:CX], -1.0 / C)
                v.tensor_mul(MUSQ[:, j:j + 1], MUn[:, j:j + 1],
                             MUn[:, j:j + 1])
                v.tensor_scalar(
                    out=SQA[:, j:j + 1], in0=SQ[:, j:j + 1],
                    scalar1=1.0 / C, scalar2=LN_EPS, op0=mult, op1=add)
                v.scalar_tensor_tensor(
                    out=VAR[:, j:j + 1], in0=SQA[:, j:j + 1], scalar=1.0,
                    in1=MUSQ[:, j:j + 1], op0=bypass, op1=subtract)
                q = qpool.tile([P, C], F32, tag="q", name=f"q{j}")
                qs[j] = q
                if ln_trivial and use_prelu:
                    s.activation(q[:], pd[:, 0:C], AF.Prelu,
                                 bias=MUn[:, j:j + 1], alpha=LEAK)
                elif ln_trivial:
                    # CoreSim fallback: Prelu isn't implemented there
                    tt = scr.tile([P, C], F32, tag="tt", name="tt")
                    s.activation(tt[:], pd[:, 0:C], AF.Identity,
                                 bias=MUn[:, j:j + 1])
                    v.scalar_tensor_tensor(
                        out=q[:], in0=tt[:], scalar=LEAK, in1=tt[:],
                        op0=mult, op1=amax)
                else:
                    # generic path: keep (diff - mu) unactivated; gamma/beta
                    # and the leaky relu are applied in finish_group
                    s.activation(q[:], pd[:, 0:C], AF.Identity,
                                 bias=MUn[:, j:j + 1])
            if ii < NT:
                i = ii
                # D' = M - x  (sign of W is pre-flipped so diff = (x-M)W^T)
                Db = scr.tile([P, C], BF16, tag="db", name=f"db{i}")
                v.scalar_tensor_tensor(
                    out=Db[:], in0=py[:, 0:C], scalar=NRS[:, i:i + 1],
                    in1=Xb[:, i, 0:C], op0=mult, op1=subtract)
                Dbs[i] = Db
            if j == 3:
                finish_group(0, 4)
            elif j in (5, 7):
                finish_group(j - 1, 2)


_PROGRAM_CACHE = {}
last_results = None


def _get_program(diag_one=True, ln_trivial=True):
    key = (diag_one, ln_trivial,
           os.environ.get("KERNEL_WARMUP", "48"),
           os.environ.get("KERNEL_WARMUP_A", "2"),
           os.environ.get("KERNEL_WARMUP_B", "2"),
           os.environ.get("KERNEL_WARMUP_C", "12"),
           os.environ.get("KERNEL_PRELU", "1"))
    if key not in _PROGRAM_CACHE:
        _PROGRAM_CACHE[key] = _build_program(diag_one, ln_trivial)
    return _PROGRAM_CACHE[key]


def _prep_inputs(local_feat, W_adj, W_aff, b_aff, ln_gamma, ln_beta):
    x = np.asarray(local_feat, np.float32).reshape(B, N, C)
    # partition-major layout with a trailing ones column: xb_r[b, p, i, :] =
    # [x[b, i*128+p, :], 1.0] -- every SBUF partition's data is one
    # contiguous DMA descriptor, and the ones column makes the G @ x matmul
    # also produce rowsum(G).
    xb = np.ones((B, P, NT, CX), dtype=BF)
    xb[:, :, :, 0:C] = x.reshape(B, NT, P, C).transpose(0, 2, 1, 3).astype(BF)
    Wf = np.asarray(W_aff, np.float32)
    wneg = np.ascontiguousarray(-Wf.T).astype(BF)        # [cin, cout]
    w1 = wneg.astype(np.float32).sum(axis=1).astype(BF)  # rowsum column
    wt = np.concatenate([wneg, w1[:, None]], axis=1)     # [C, 257] bf16
    wt_r = np.ascontiguousarray(
        wt.reshape(CT, P, CX).transpose(1, 0, 2))        # [P, CT, 257]
    identb = np.eye(P, dtype=np.float32).astype(BF)
    diag = np.ascontiguousarray(np.diagonal(np.asarray(W_adj, np.float32)))
    wd = np.ascontiguousarray(diag.reshape(CT, P).T).astype(np.float32)
    b = np.asarray(b_aff, np.float32).ravel()
    g = np.asarray(ln_gamma, np.float32).ravel()
    be = np.asarray(ln_beta, np.float32).ravel()
    rows = np.concatenate([g, be]).reshape(1, 2 * C).astype(np.float32)
    bvec = np.concatenate([b, [b.sum()]]).reshape(1, CX).astype(BF)
    diag_one = bool(np.all(diag == 1.0))
    ln_trivial = bool(np.all(g == 1.0) and np.all(be == 0.0)
                      and np.all(b == 0.0))
    in_maps = [
        {"xb": np.ascontiguousarray(xb[bb]), "wt": wt_r, "identb": identb,
         "wdiag": wd, "rows": rows, "bvec": bvec}
        for bb in range(B)
    ]
    return in_maps, diag_one, ln_trivial


def kernel(local_feat, global_feat, pos, W_adj, W_aff, b_aff, ln_gamma,
           ln_beta, **_unused):
    global last_results
    in_maps, diag_one, ln_trivial = _prep_inputs(
        local_feat, W_adj, W_aff, b_aff, ln_gamma, ln_beta)
    nc = _get_program(diag_one, ln_trivial)
    trace = bool(int(os.environ.get("KERNEL_TRACE", "0")))
    res = run_bass_kernel_spmd(nc, in_maps, list(range(B)), trace=trace)
    last_results = res
    out = np.stack([np.asarray(res.results[bb]["y"]) for bb in range(B)],
                   axis=0)                                # [B, P, NT, C]
    out = out.transpose(0, 2, 1, 3).reshape(B, N, C)      # n = i*128 + p
    return out.reshape(B, T, NN, C).astype(np.float32)
